# revision 8
# baseline (speedup 1.0000x reference)
"""Causal multi-head attention on 8 Trainium2 NeuronCores.

Problem: B=4, S=2048, E=2048, H=16 heads (HD=128), fp32 I/O.

Sharding (tensor-parallel on heads + sequence-parallel out-proj):
  - Every core holds the full (host-transposed, bf16-cast) activations and
    projects Q/K/V only for its 2 heads (per-core slices of Wq/Wk/Wv rows).
  - Attention (scores -> exp -> normalize -> @V) runs fully local per
    (batch, head), producing attn_outT [d_local=256, s=2048] per batch.
  - An AllToAll redistributes attn_outT from head-sharded to
    sequence-sharded: core c ends with attn_outT [e=2048, s_c=256] per batch.
  - Out-projection is computed for the core's 256 sequence rows per batch;
    the host concatenates row-slices - no further reduction needed.

v2 design notes (from perfetto analysis of v1 @ 1011us):
  - The PE only reaches its full 2.4GHz clock after ~3us of continuous
    execution; every idle gap drops it to 1.2GHz. So the whole kernel is
    organized to keep the PE stream dense:
      * proj uses one mega-DMA [128,16,512] per 512-col block (sync queue,
        ~12 issues/batch instead of ~96) and 2 serially-accumulated psum
        banks, leaving banks for overlap.
      * psum pools: scores 2 + attn-acc 2 + den 1 + proj/outproj 3 = 8.
      * outproj(b-1) is emitted interleaved after each attention span of
        batch b, so the out-of-order Tile scheduler uses its matmuls to
        fill PE slack in the EXP-paced attention phase. proj(b+1) fills
        whatever is left (it is emitted later = lower priority).
  - The v1 softmax epilogue (DVE reciprocal 3.3us + den DMA + ao-mul all
    serialized on the scalar queue) stalled the exp stream ~5us/span-pair.
    Now: reciprocal_approx_fast (DVE, ~0.7us, 18-bit - plenty for bf16
    outputs), den row-1 hop + broadcasts + acc evictions on gpsimd, ao
    muls + output DMAs on the vector queue. The scalar queue runs exps
    back-to-back only.
  - Wo is no longer SBUF-resident (64KB/partition): it streams per 512-col
    slice during the attention window (DMA is idle there). Frees SBUF for
    x-block triple buffering.
"""

import numpy as np
import ml_dtypes

import concourse.bacc as bacc
import concourse.mybir as mybir
import concourse.tile as tile
import concourse.bass_utils as bass_utils
from concourse.masks import make_identity

B, S, E, H = 4, 2048, 2048, 16
HD = E // H            # 128
N_CORES = 8
H_LOC = H // N_CORES   # 2 heads per core
F_LOC = H_LOC * HD     # 256 features per core (head slice)
S_LOC = S // N_CORES   # 256 sequence rows per core (out-proj slice)
P = 128
NS = 512               # matmul free-dim span
EC = E // P            # 16 contraction chunks
QSP = S // NS          # 4 q-spans per (b, h)
KCH = S // P           # 16 k-chunks
NBLK = S // NS         # 4 proj blocks per tensor
NF = E // NS           # 4 out-proj feature spans
INV_SQRT_HD = float(1.0 / np.sqrt(HD))

BF16 = mybir.dt.bfloat16
F32 = mybir.dt.float32

_cached_nc = None


def _build():
    nc = bacc.Bacc("TRN2", target_bir_lowering=False, debug=False,
                   num_devices=N_CORES)

    # ---------------- I/O ----------------
    qt_d = nc.dram_tensor("qt", [B, E, S], BF16, kind="ExternalInput")
    kt_d = nc.dram_tensor("kt", [B, E, S], BF16, kind="ExternalInput")
    vt_d = nc.dram_tensor("vt", [B, E, S], BF16, kind="ExternalInput")
    wqt_d = nc.dram_tensor("wqt", [E, F_LOC], BF16, kind="ExternalInput")
    wkt_d = nc.dram_tensor("wkt", [E, F_LOC], BF16, kind="ExternalInput")
    wvt_d = nc.dram_tensor("wvt", [E, F_LOC], BF16, kind="ExternalInput")
    wot_d = nc.dram_tensor("wot", [E, E], BF16, kind="ExternalInput")
    bias_d = nc.dram_tensor("bias_bc", [P, E], BF16, kind="ExternalInput")
    masks_d = nc.dram_tensor("masks", [4, P, NS], BF16, kind="ExternalInput")
    out_d = nc.dram_tensor("out", [B, S_LOC, E], F32, kind="ExternalOutput")

    wot_v = wot_d.ap().rearrange("(ec p) f -> p ec f", p=P)

    with tile.TileContext(nc) as tc:
        with (
            tc.tile_pool(name="wconst", bufs=1) as wconst,
            tc.tile_pool(name="proj", bufs=2) as proj,
            tc.tile_pool(name="xs", bufs=3) as xs,
            tc.tile_pool(name="wop", bufs=2) as wop,
            tc.tile_pool(name="lhsp", bufs=2) as lhsp,
            tc.tile_pool(name="expp", bufs=8) as expp,
            tc.tile_pool(name="smallp", bufs=2) as smallp,
            tc.tile_pool(name="outp", bufs=2) as outp,
            tc.tile_pool(name="ps_s", bufs=2, space="PSUM") as ps_s,
            tc.tile_pool(name="ps_acc", bufs=2, space="PSUM") as ps_acc,
            tc.tile_pool(name="ps_den", bufs=1, space="PSUM") as ps_den,
            tc.tile_pool(name="ps_po", bufs=3, space="PSUM") as ps_po,
            tc.tile_pool(name="dram", bufs=1, space="DRAM") as dram,
        ):
            # ------------ constants / weights resident in SBUF ------------
            wq_sb = wconst.tile([P, EC, F_LOC], BF16, tag="wq")
            wk_sb = wconst.tile([P, EC, F_LOC], BF16, tag="wk")
            wv_sb = wconst.tile([P, EC, F_LOC], BF16, tag="wv")
            nc.sync.dma_start(wq_sb[:], wqt_d.ap().rearrange("(ec p) f -> p ec f", p=P))
            nc.sync.dma_start(wk_sb[:], wkt_d.ap().rearrange("(ec p) f -> p ec f", p=P))
            nc.sync.dma_start(wv_sb[:], wvt_d.ap().rearrange("(ec p) f -> p ec f", p=P))
            bias_sb = wconst.tile([P, E], BF16, tag="bias")
            nc.scalar.dma_start(bias_sb[:], bias_d.ap())
            mask_sb = wconst.tile([P, 4, NS], BF16, tag="mask")
            nc.scalar.dma_start(mask_sb[:], masks_d.ap().rearrange("r p q -> p r q"))
            # one-hot [P, H_LOC] lhsTs: column h all-ones, other column zero -
            # the denominator matmul for head h lands in psum row h.
            onehot_sb = []
            for h in range(H_LOC):
                t = wconst.tile([P, H_LOC], BF16, tag=f"onehot{h}",
                                name=f"onehot{h}")
                nc.vector.memset(t[:], 0.0)
                nc.vector.memset(t[:, h:h + 1], 1.0)
                onehot_sb.append(t)
            ident_sb = wconst.tile([P, P], BF16, tag="ident")
            make_identity(nc, ident_sb[:])

            a2a_in = [dram.tile([N_CORES, F_LOC, S_LOC], BF16,
                                tag=f"a2a_in{b}", name=f"a2a_in{b}")
                      for b in range(B)]
            a2a_out = [dram.tile([N_CORES, F_LOC, S_LOC], BF16,
                                 tag=f"a2a_out{b}", name=f"a2a_out{b}")
                       for b in range(B)]

            # out-proj lhs tiles, keyed by batch (loaded lazily at nf==0);
            # pending (psum, b, nf, sc) evictions, flushed one span later so
            # the DVE never queue-blocks on in-flight filler matmuls.
            lhs_tiles = {}
            pending_evict = []

            def flush_evicts():
                while pending_evict:
                    ps, eb, enf, esc = pending_evict.pop(0)
                    o_t = outp.tile([P, NS], F32, tag="o", name="o_t")
                    nc.vector.tensor_add(o_t[:], ps[:],
                                         bias_sb[:, enf * NS:(enf + 1) * NS])
                    nc.gpsimd.dma_start(
                        out_d.ap()[eb, esc * P:(esc + 1) * P,
                                   enf * NS:(enf + 1) * NS],
                        o_t[:])

            def outproj_nf(b, nf):
                """One 512-wide feature span of batch b's out-projection.

                Emitted interleaved between attention spans of batch b+1 so
                its matmuls fill PE slack there. Evictions are deferred to
                the next call (the matmuls have completed by then).
                """
                flush_evicts()
                if nf == 0:
                    lts = []
                    for sc in range(S_LOC // P):
                        l_t = lhsp.tile([P, EC, P], BF16, tag="lo", name="lo_t")
                        nc.gpsimd.dma_start(
                            l_t[:],
                            a2a_out[b][:, :, sc * P:(sc + 1) * P]
                            .rearrange("r (dc p) s -> p (r dc) s", p=P))
                        lts.append(l_t)
                    lhs_tiles[b] = lts
                wo_t = wop.tile([P, EC, NS], BF16, tag="wo")
                nc.sync.dma_start(wo_t[:], wot_v[:, :, nf * NS:(nf + 1) * NS])
                for sc in range(S_LOC // P):
                    ps = ps_po.tile([P, NS], F32, tag="po", name="ops")
                    for ec in range(EC):
                        nc.tensor.matmul(ps[:], lhs_tiles[b][sc][:, ec, :],
                                         wo_t[:, ec, :],
                                         start=(ec == 0), stop=(ec == EC - 1))
                    pending_evict.append((ps, b, nf, sc))

            for b in range(B):
                # -------- Q/K/V projections, all in T-layout [d, s] -------
                # One mega-DMA [128, EC, 512] per 512-col block; per head a
                # single psum bank accumulates serially over all 16 e-chunks.
                qT_sb = proj.tile([P, H_LOC, S], BF16, tag="qT")
                kT_sb = proj.tile([P, H_LOC, S], BF16, tag="kT")
                vT_sb = proj.tile([P, H_LOC, S], BF16, tag="vT", bufs=1)
                v_sb = proj.tile([P, KCH, F_LOC], BF16, tag="v", bufs=1)

                for src_d, w_sb, dst in (
                        (qt_d, wq_sb, qT_sb),
                        (kt_d, wk_sb, kT_sb),
                        (vt_d, wv_sb, vT_sb)):
                    src_v = src_d.ap()[b].rearrange("(ec p) s -> p ec s", p=P)
                    for blk in range(NBLK):
                        x_t = xs.tile([P, EC, NS], BF16, tag="x")
                        nc.sync.dma_start(x_t[:],
                                          src_v[:, :, blk * NS:(blk + 1) * NS])
                        for h in range(H_LOC):
                            ps = ps_po.tile([P, NS], F32, tag="po", name="pps")
                            for ec in range(EC):
                                nc.tensor.matmul(
                                    ps[:], w_sb[:, ec, h * HD:(h + 1) * HD],
                                    x_t[:, ec, :],
                                    start=(ec == 0), stop=(ec == EC - 1))
                            nc.vector.tensor_copy(
                                dst[:, h, blk * NS:(blk + 1) * NS], ps[:])

                # v [s, d] from vT via PE transposes
                for sc in range(KCH):
                    for h in range(H_LOC):
                        tps = ps_po.tile([P, P], BF16, tag="po", name="tps")
                        nc.tensor.transpose(tps[:], vT_sb[:, h, sc * P:(sc + 1) * P],
                                            ident_sb[:])
                        nc.vector.tensor_copy(v_sb[:, sc, h * HD:(h + 1) * HD],
                                              tps[:])

                # ----- attention: q-span outer, head inner; the two heads'
                # denominators pack into one [2, NS] psum via one-hot lhsT ----
                for i in range(QSP):
                    den_ps = ps_den.tile([H_LOC, NS], F32, tag="den")
                    aof_list = []
                    n_k = 4 * i + 4
                    for h in range(H_LOC):
                        outT_ps = ps_acc.tile([P, NS], F32, tag="acc",
                                              name=f"acc{h}")
                        dacc = expp.tile([P, NS], BF16, tag="dacc", bufs=2)
                        for j in range(n_k):
                            s_ps = ps_s.tile([P, NS], F32, tag="s")
                            nc.tensor.matmul(
                                s_ps[:], kT_sb[:, h, j * P:(j + 1) * P],
                                qT_sb[:, h, i * NS:(i + 1) * NS],
                                start=True, stop=True)
                            e_t = expp.tile([P, NS], BF16, tag="e", bufs=8)
                            nc.scalar.activation(e_t[:], s_ps[:],
                                                 mybir.ActivationFunctionType.Exp,
                                                 scale=INV_SQRT_HD)
                            r = j - 4 * i
                            if r >= 0:
                                nc.vector.tensor_mul(e_t[:], e_t[:], mask_sb[:, r, :])
                            # denominator partials accumulate in bf16, h0's
                            # chain on DVE and h1's on gpsimd (SBUF-only, so
                            # gpsimd is legal) to split the per-iter load
                            deng = nc.vector if h == 0 else nc.gpsimd
                            if j == 0:
                                deng.tensor_copy(dacc[:], e_t[:])
                            else:
                                deng.tensor_add(dacc[:], dacc[:], e_t[:])
                            nc.tensor.matmul(outT_ps[:], v_sb[:, j, h * HD:(h + 1) * HD],
                                             e_t[:], start=(j == 0), stop=(j == n_k - 1))
                        # fold the 128 partitions of dacc into psum row h
                        nc.tensor.matmul(den_ps[:], onehot_sb[h][:], dacc[:],
                                         start=(h == 0), stop=(h == H_LOC - 1))
                        # evict the accumulator promptly so the psum bank
                        # frees without waiting on the reciprocal chain
                        aof = smallp.tile([P, NS], BF16, tag="aof", bufs=3,
                                          name="aof")
                        nc.vector.tensor_copy(aof[:], outT_ps[:])
                        aof_list.append(aof)
                    den_rec = smallp.tile([H_LOC, NS], F32, tag="den_rec")
                    nc.vector.reciprocal_approx_fast(den_rec[:], den_ps[:])
                    # partition_broadcast only reads partition 0: move row 1 down
                    den_r1 = smallp.tile([1, NS], F32, tag="den_r1")
                    nc.gpsimd.dma_start(den_r1[:], den_rec[1:2, :])
                    for h in range(H_LOC):
                        den_bc = smallp.tile([P, NS], F32, tag="den_bc")
                        nc.gpsimd.partition_broadcast(
                            den_bc[:], den_rec[0:1, :] if h == 0 else den_r1[:])
                        ao = smallp.tile([P, NS], BF16, tag="ao")
                        nc.vector.tensor_mul(ao[:], aof_list[h][:], den_bc[:])
                        dst = a2a_in[b][2 * i:2 * i + 2, h * HD:(h + 1) * HD, :]
                        nc.sync.dma_start(dst.transpose([1, 0, 2]),
                                          ao[:].rearrange("p (g q) -> p g q", g=2))
                    # out-projection of the PREVIOUS batch, one feature span
                    # per attention span: ready PE filler for the exp-paced
                    # attention window, at lower scheduler priority.
                    if b > 0:
                        outproj_nf(b - 1, i)

                # ---------------- head -> sequence redistribution ---------
                nc.gpsimd.collective_compute(
                    "AllToAll", mybir.AluOpType.bypass,
                    replica_groups=[list(range(N_CORES))],
                    ins=[a2a_in[b][:].opt()], outs=[a2a_out[b][:].opt()])

            for nf in range(NF):
                outproj_nf(B - 1, nf)
            flush_evicts()

    nc.compile()
    return nc


def _get_nc():
    global _cached_nc
    if _cached_nc is None:
        _cached_nc = _build()
    return _cached_nc


def kernel(query, key, value, key_padding_mask, Wq, Wk, Wv, Wo, bo):
    query = np.asarray(query, dtype=np.float32)
    key = np.asarray(key, dtype=np.float32)
    value = np.asarray(value, dtype=np.float32)
    Wq = np.asarray(Wq, dtype=np.float32)
    Wk = np.asarray(Wk, dtype=np.float32)
    Wv = np.asarray(Wv, dtype=np.float32)
    Wo = np.asarray(Wo, dtype=np.float32)
    bo = np.asarray(bo, dtype=np.float32)

    bf = ml_dtypes.bfloat16
    # host-side layout prep: transpose activations to [b, e, s], cast to bf16
    qt = np.ascontiguousarray(query.transpose(0, 2, 1)).astype(bf)
    kt = np.ascontiguousarray(key.transpose(0, 2, 1)).astype(bf)
    vt = np.ascontiguousarray(value.transpose(0, 2, 1)).astype(bf)
    wot = np.ascontiguousarray(Wo.T).astype(bf)
    bias_bc = np.broadcast_to(bo, (P, E)).astype(bf)

    # causal masks for the 4 diagonal shifts: mask_r[kk, qq] = kk <= qq - 128 r
    kk = np.arange(P)[:, None]
    qq = np.arange(NS)[None, :]
    masks = np.stack([(kk <= qq - P * r) for r in range(4)]).astype(bf)

    in_maps = []
    for c in range(N_CORES):
        sl = slice(c * F_LOC, (c + 1) * F_LOC)
        in_maps.append(dict(
            qt=qt, kt=kt, vt=vt,
            wqt=np.ascontiguousarray(Wq[sl].T).astype(bf),
            wkt=np.ascontiguousarray(Wk[sl].T).astype(bf),
            wvt=np.ascontiguousarray(Wv[sl].T).astype(bf),
            wot=wot, bias_bc=bias_bc, masks=masks,
        ))

    nc = _get_nc()
    res = bass_utils.run_bass_kernel_spmd(
        nc, in_maps, core_ids=list(range(N_CORES)), trace=False)

    out = np.empty((B, S, E), dtype=np.float32)
    for c in range(N_CORES):
        out[:, c * S_LOC:(c + 1) * S_LOC, :] = res.results[c]["out"]
    return out


# revision 9
# speedup vs baseline: 1.3433x; 1.3433x over previous
"""Causal multi-head attention on 8 Trainium2 NeuronCores.

Problem: B=4, S=2048, E=2048, H=16 heads (HD=128), fp32 I/O.

Sharding (tensor-parallel on heads + sequence-parallel out-proj):
  - Every core holds the full (host-transposed, bf16-cast) activations and
    projects Q/K/V only for its 2 heads (per-core slices of Wq/Wk/Wv rows).
  - Attention (scores -> exp -> normalize -> @V) runs fully local per
    (batch, head), producing attn_outT [d_local=256, s=2048] per batch.
  - An AllToAll redistributes attn_outT from head-sharded to
    sequence-sharded: core c ends with attn_outT [e=2048, s_c=256] per batch.
  - Out-projection is computed for the core's 256 sequence rows per batch;
    the host concatenates row-slices - no further reduction needed.

v2 design notes (from perfetto analysis of v1 @ 1011us):
  - The PE only reaches its full 2.4GHz clock after ~3us of continuous
    execution; every idle gap drops it to 1.2GHz. So the whole kernel is
    organized to keep the PE stream dense:
      * proj uses one mega-DMA [128,16,512] per 512-col block (sync queue,
        ~12 issues/batch instead of ~96) and 2 serially-accumulated psum
        banks, leaving banks for overlap.
      * psum pools: scores 2 + attn-acc 2 + den 1 + proj/outproj 3 = 8.
      * outproj(b-1) is emitted interleaved after each attention span of
        batch b, so the out-of-order Tile scheduler uses its matmuls to
        fill PE slack in the EXP-paced attention phase. proj(b+1) fills
        whatever is left (it is emitted later = lower priority).
  - The v1 softmax epilogue (DVE reciprocal 3.3us + den DMA + ao-mul all
    serialized on the scalar queue) stalled the exp stream ~5us/span-pair.
    Now: reciprocal_approx_fast (DVE, ~0.7us, 18-bit - plenty for bf16
    outputs), den row-1 hop + broadcasts + acc evictions on gpsimd, ao
    muls + output DMAs on the vector queue. The scalar queue runs exps
    back-to-back only.
  - Wo is no longer SBUF-resident (64KB/partition): it streams per 512-col
    slice during the attention window (DMA is idle there). Frees SBUF for
    x-block triple buffering.
"""

import numpy as np
import ml_dtypes

import concourse.bacc as bacc
import concourse.mybir as mybir
import concourse.tile as tile
import concourse.bass_utils as bass_utils
from concourse.masks import make_identity

B, S, E, H = 4, 2048, 2048, 16
HD = E // H            # 128
N_CORES = 8
H_LOC = H // N_CORES   # 2 heads per core
F_LOC = H_LOC * HD     # 256 features per core (head slice)
S_LOC = S // N_CORES   # 256 sequence rows per core (out-proj slice)
P = 128
NS = 512               # matmul free-dim span
EC = E // P            # 16 contraction chunks
QSP = S // NS          # 4 q-spans per (b, h)
KCH = S // P           # 16 k-chunks
NBLK = S // NS         # 4 proj blocks per tensor
NF = E // NS           # 4 out-proj feature spans
INV_SQRT_HD = float(1.0 / np.sqrt(HD))

BF16 = mybir.dt.bfloat16
F32 = mybir.dt.float32

_cached_nc = None


def _build():
    nc = bacc.Bacc("TRN2", target_bir_lowering=False, debug=False,
                   num_devices=N_CORES)

    # ---------------- I/O ----------------
    qt_d = nc.dram_tensor("qt", [B, E, S], BF16, kind="ExternalInput")
    kt_d = nc.dram_tensor("kt", [B, E, S], BF16, kind="ExternalInput")
    vt_d = nc.dram_tensor("vt", [B, E, S], BF16, kind="ExternalInput")
    wqt_d = nc.dram_tensor("wqt", [E, F_LOC], BF16, kind="ExternalInput")
    wkt_d = nc.dram_tensor("wkt", [E, F_LOC], BF16, kind="ExternalInput")
    wvt_d = nc.dram_tensor("wvt", [E, F_LOC], BF16, kind="ExternalInput")
    wot_d = nc.dram_tensor("wot", [E, E], BF16, kind="ExternalInput")
    bias_d = nc.dram_tensor("bias_bc", [P, E], BF16, kind="ExternalInput")
    masks_d = nc.dram_tensor("masks", [4, P, NS], BF16, kind="ExternalInput")
    out_d = nc.dram_tensor("out", [B, S_LOC, E], F32, kind="ExternalOutput")

    wot_v = wot_d.ap().rearrange("(ec p) f -> p ec f", p=P)

    with tile.TileContext(nc) as tc:
        with (
            tc.tile_pool(name="wconst", bufs=1) as wconst,
            tc.tile_pool(name="proj", bufs=2) as proj,
            tc.tile_pool(name="xs", bufs=3) as xs,
            tc.tile_pool(name="wop", bufs=2) as wop,
            tc.tile_pool(name="lhsp", bufs=2) as lhsp,
            tc.tile_pool(name="expp", bufs=8) as expp,
            tc.tile_pool(name="smallp", bufs=2) as smallp,
            tc.tile_pool(name="outp", bufs=2) as outp,
            tc.tile_pool(name="ps_s", bufs=2, space="PSUM") as ps_s,
            tc.tile_pool(name="ps_acc", bufs=2, space="PSUM") as ps_acc,
            tc.tile_pool(name="ps_den", bufs=1, space="PSUM") as ps_den,
            tc.tile_pool(name="ps_po", bufs=3, space="PSUM") as ps_po,
            tc.tile_pool(name="dram", bufs=1, space="DRAM") as dram,
        ):
            # ------------ constants / weights resident in SBUF ------------
            wq_sb = wconst.tile([P, EC, F_LOC], BF16, tag="wq")
            wk_sb = wconst.tile([P, EC, F_LOC], BF16, tag="wk")
            wv_sb = wconst.tile([P, EC, F_LOC], BF16, tag="wv")
            nc.sync.dma_start(wq_sb[:], wqt_d.ap().rearrange("(ec p) f -> p ec f", p=P))
            nc.sync.dma_start(wk_sb[:], wkt_d.ap().rearrange("(ec p) f -> p ec f", p=P))
            nc.sync.dma_start(wv_sb[:], wvt_d.ap().rearrange("(ec p) f -> p ec f", p=P))
            bias_sb = wconst.tile([P, E], BF16, tag="bias")
            nc.scalar.dma_start(bias_sb[:], bias_d.ap())
            mask_sb = wconst.tile([P, 4, NS], BF16, tag="mask")
            nc.scalar.dma_start(mask_sb[:], masks_d.ap().rearrange("r p q -> p r q"))
            # one-hot [P, H_LOC] lhsTs: column h all-ones, other column zero -
            # the denominator matmul for head h lands in psum row h.
            onehot_sb = []
            for h in range(H_LOC):
                t = wconst.tile([P, H_LOC], BF16, tag=f"onehot{h}",
                                name=f"onehot{h}")
                nc.vector.memset(t[:], 0.0)
                nc.vector.memset(t[:, h:h + 1], 1.0)
                onehot_sb.append(t)
            ident_sb = wconst.tile([P, P], BF16, tag="ident")
            make_identity(nc, ident_sb[:])

            a2a_in = [dram.tile([N_CORES, F_LOC, S_LOC], BF16,
                                tag=f"a2a_in{b}", name=f"a2a_in{b}")
                      for b in range(B)]
            a2a_out = [dram.tile([N_CORES, F_LOC, S_LOC], BF16,
                                 tag=f"a2a_out{b}", name=f"a2a_out{b}")
                       for b in range(B)]

            # out-proj lhs tiles, keyed by batch (loaded lazily at nf==0);
            # pending (psum, b, nf, sc) evictions, flushed one span later so
            # the DVE never queue-blocks on in-flight filler matmuls.
            lhs_tiles = {}
            pending_evict = []

            def flush_evicts():
                while pending_evict:
                    ps, eb, enf, esc = pending_evict.pop(0)
                    o_t = outp.tile([P, NS], F32, tag="o", name="o_t")
                    nc.vector.tensor_add(o_t[:], ps[:],
                                         bias_sb[:, enf * NS:(enf + 1) * NS])
                    nc.gpsimd.dma_start(
                        out_d.ap()[eb, esc * P:(esc + 1) * P,
                                   enf * NS:(enf + 1) * NS],
                        o_t[:])

            def outproj_nf(b, nf):
                """One 512-wide feature span of batch b's out-projection.

                Emitted interleaved between attention spans of batch b+1 so
                its matmuls fill PE slack there. Evictions are deferred to
                the next call (the matmuls have completed by then).
                """
                flush_evicts()
                if nf == 0:
                    lts = []
                    for sc in range(S_LOC // P):
                        l_t = lhsp.tile([P, EC, P], BF16, tag="lo", name="lo_t")
                        nc.gpsimd.dma_start(
                            l_t[:],
                            a2a_out[b][:, :, sc * P:(sc + 1) * P]
                            .rearrange("r (dc p) s -> p (r dc) s", p=P))
                        lts.append(l_t)
                    lhs_tiles[b] = lts
                wo_t = wop.tile([P, EC, NS], BF16, tag="wo")
                nc.sync.dma_start(wo_t[:], wot_v[:, :, nf * NS:(nf + 1) * NS])
                for sc in range(S_LOC // P):
                    ps = ps_po.tile([P, NS], F32, tag="po", name="ops")
                    for ec in range(EC):
                        nc.tensor.matmul(ps[:], lhs_tiles[b][sc][:, ec, :],
                                         wo_t[:, ec, :],
                                         start=(ec == 0), stop=(ec == EC - 1))
                    pending_evict.append((ps, b, nf, sc))

            for b in range(B):
                # -------- Q/K/V projections, all in T-layout [d, s] -------
                # One mega-DMA [128, EC, 512] per 512-col block; per head a
                # single psum bank accumulates serially over all 16 e-chunks.
                qT_sb = proj.tile([P, H_LOC, S], BF16, tag="qT")
                kT_sb = proj.tile([P, H_LOC, S], BF16, tag="kT")
                vT_sb = proj.tile([P, H_LOC, S], BF16, tag="vT", bufs=1)
                v_sb = proj.tile([P, KCH, F_LOC], BF16, tag="v", bufs=1)

                for src_d, w_sb, dst in (
                        (qt_d, wq_sb, qT_sb),
                        (kt_d, wk_sb, kT_sb),
                        (vt_d, wv_sb, vT_sb)):
                    src_v = src_d.ap()[b].rearrange("(ec p) s -> p ec s", p=P)
                    for blk in range(NBLK):
                        x_t = xs.tile([P, EC, NS], BF16, tag="x")
                        nc.sync.dma_start(x_t[:],
                                          src_v[:, :, blk * NS:(blk + 1) * NS])
                        for h in range(H_LOC):
                            ps = ps_po.tile([P, NS], F32, tag="po", name="pps")
                            for ec in range(EC):
                                nc.tensor.matmul(
                                    ps[:], w_sb[:, ec, h * HD:(h + 1) * HD],
                                    x_t[:, ec, :],
                                    start=(ec == 0), stop=(ec == EC - 1))
                            nc.vector.tensor_copy(
                                dst[:, h, blk * NS:(blk + 1) * NS], ps[:])

                # v [s, d] from vT via PE transposes
                for sc in range(KCH):
                    for h in range(H_LOC):
                        tps = ps_po.tile([P, P], BF16, tag="po", name="tps")
                        nc.tensor.transpose(tps[:], vT_sb[:, h, sc * P:(sc + 1) * P],
                                            ident_sb[:])
                        nc.vector.tensor_copy(v_sb[:, sc, h * HD:(h + 1) * HD],
                                              tps[:])

                # ----- attention: q-span outer, head inner; the two heads'
                # denominators pack into one [2, NS] psum via one-hot lhsT ----
                for i in range(QSP):
                    den_ps = ps_den.tile([H_LOC, NS], F32, tag="den")
                    aof_list = []
                    n_k = 4 * i + 4
                    for h in range(H_LOC):
                        outT_ps = ps_acc.tile([P, NS], F32, tag="acc",
                                              name=f"acc{h}")
                        dacc = expp.tile([P, NS], BF16, tag="dacc", bufs=2)
                        for j in range(n_k):
                            s_ps = ps_s.tile([P, NS], F32, tag="s")
                            nc.tensor.matmul(
                                s_ps[:], kT_sb[:, h, j * P:(j + 1) * P],
                                qT_sb[:, h, i * NS:(i + 1) * NS],
                                start=True, stop=True)
                            e_t = expp.tile([P, NS], BF16, tag="e", bufs=8)
                            nc.scalar.activation(e_t[:], s_ps[:],
                                                 mybir.ActivationFunctionType.Exp,
                                                 scale=INV_SQRT_HD)
                            r = j - 4 * i
                            if r >= 0:
                                nc.vector.tensor_mul(e_t[:], e_t[:], mask_sb[:, r, :])
                            # denominator partials accumulate on DVE (bf16)
                            if j == 0:
                                nc.vector.tensor_copy(dacc[:], e_t[:])
                            else:
                                nc.vector.tensor_add(dacc[:], dacc[:], e_t[:])
                            nc.tensor.matmul(outT_ps[:], v_sb[:, j, h * HD:(h + 1) * HD],
                                             e_t[:], start=(j == 0), stop=(j == n_k - 1))
                        # fold the 128 partitions of dacc into psum row h
                        nc.tensor.matmul(den_ps[:], onehot_sb[h][:], dacc[:],
                                         start=(h == 0), stop=(h == H_LOC - 1))
                        # evict the accumulator promptly so the psum bank
                        # frees without waiting on the reciprocal chain
                        aof = smallp.tile([P, NS], BF16, tag="aof", bufs=3,
                                          name="aof")
                        nc.vector.tensor_copy(aof[:], outT_ps[:])
                        aof_list.append(aof)
                    den_rec = smallp.tile([H_LOC, NS], F32, tag="den_rec")
                    nc.vector.reciprocal_approx_fast(den_rec[:], den_ps[:])
                    # partition_broadcast only reads partition 0: move row 1 down
                    den_r1 = smallp.tile([1, NS], F32, tag="den_r1")
                    nc.gpsimd.dma_start(den_r1[:], den_rec[1:2, :])
                    for h in range(H_LOC):
                        den_bc = smallp.tile([P, NS], F32, tag="den_bc")
                        nc.gpsimd.partition_broadcast(
                            den_bc[:], den_rec[0:1, :] if h == 0 else den_r1[:])
                        ao = smallp.tile([P, NS], BF16, tag="ao")
                        nc.vector.tensor_mul(ao[:], aof_list[h][:], den_bc[:])
                        dst = a2a_in[b][2 * i:2 * i + 2, h * HD:(h + 1) * HD, :]
                        nc.sync.dma_start(dst.transpose([1, 0, 2]),
                                          ao[:].rearrange("p (g q) -> p g q", g=2))
                    # out-projection of the PREVIOUS batch, one feature span
                    # per attention span: ready PE filler for the exp-paced
                    # attention window, at lower scheduler priority.
                    if b > 0:
                        outproj_nf(b - 1, i)

                # ---------------- head -> sequence redistribution ---------
                nc.gpsimd.collective_compute(
                    "AllToAll", mybir.AluOpType.bypass,
                    replica_groups=[list(range(N_CORES))],
                    ins=[a2a_in[b][:].opt()], outs=[a2a_out[b][:].opt()])

            for nf in range(NF):
                outproj_nf(B - 1, nf)
            flush_evicts()

    nc.compile()
    return nc


def _get_nc():
    global _cached_nc
    if _cached_nc is None:
        _cached_nc = _build()
    return _cached_nc


def kernel(query, key, value, key_padding_mask, Wq, Wk, Wv, Wo, bo):
    query = np.asarray(query, dtype=np.float32)
    key = np.asarray(key, dtype=np.float32)
    value = np.asarray(value, dtype=np.float32)
    Wq = np.asarray(Wq, dtype=np.float32)
    Wk = np.asarray(Wk, dtype=np.float32)
    Wv = np.asarray(Wv, dtype=np.float32)
    Wo = np.asarray(Wo, dtype=np.float32)
    bo = np.asarray(bo, dtype=np.float32)

    bf = ml_dtypes.bfloat16
    # host-side layout prep: transpose activations to [b, e, s], cast to bf16
    qt = np.ascontiguousarray(query.transpose(0, 2, 1)).astype(bf)
    kt = np.ascontiguousarray(key.transpose(0, 2, 1)).astype(bf)
    vt = np.ascontiguousarray(value.transpose(0, 2, 1)).astype(bf)
    wot = np.ascontiguousarray(Wo.T).astype(bf)
    bias_bc = np.broadcast_to(bo, (P, E)).astype(bf)

    # causal masks for the 4 diagonal shifts: mask_r[kk, qq] = kk <= qq - 128 r
    kk = np.arange(P)[:, None]
    qq = np.arange(NS)[None, :]
    masks = np.stack([(kk <= qq - P * r) for r in range(4)]).astype(bf)

    in_maps = []
    for c in range(N_CORES):
        sl = slice(c * F_LOC, (c + 1) * F_LOC)
        in_maps.append(dict(
            qt=qt, kt=kt, vt=vt,
            wqt=np.ascontiguousarray(Wq[sl].T).astype(bf),
            wkt=np.ascontiguousarray(Wk[sl].T).astype(bf),
            wvt=np.ascontiguousarray(Wv[sl].T).astype(bf),
            wot=wot, bias_bc=bias_bc, masks=masks,
        ))

    nc = _get_nc()
    res = bass_utils.run_bass_kernel_spmd(
        nc, in_maps, core_ids=list(range(N_CORES)), trace=False)

    out = np.empty((B, S, E), dtype=np.float32)
    for c in range(N_CORES):
        out[:, c * S_LOC:(c + 1) * S_LOC, :] = res.results[c]["out"]
    return out


# revision 14
# speedup vs baseline: 1.3478x; 1.0033x over previous
"""Causal multi-head attention on 8 Trainium2 NeuronCores.

Problem: B=4, S=2048, E=2048, H=16 heads (HD=128), fp32 I/O.

Sharding (tensor-parallel on heads + sequence-parallel out-proj):
  - Every core holds the full (host-transposed, bf16-cast) activations and
    projects Q/K/V only for its 2 heads (per-core slices of Wq/Wk/Wv rows).
  - Attention (scores -> exp -> normalize -> @V) runs fully local per
    (batch, head), producing attn_outT [d_local=256, s=2048] per batch.
  - An AllToAll redistributes attn_outT from head-sharded to
    sequence-sharded: core c ends with attn_outT [e=2048, s_c=256] per batch.
  - Out-projection is computed for the core's 256 sequence rows per batch;
    the host concatenates row-slices - no further reduction needed.

v2 design notes (from perfetto analysis of v1 @ 1011us):
  - The PE only reaches its full 2.4GHz clock after ~3us of continuous
    execution; every idle gap drops it to 1.2GHz. So the whole kernel is
    organized to keep the PE stream dense:
      * proj uses one mega-DMA [128,16,512] per 512-col block (sync queue,
        ~12 issues/batch instead of ~96) and 2 serially-accumulated psum
        banks, leaving banks for overlap.
      * psum pools: scores 2 + attn-acc 2 + den 1 + proj/outproj 3 = 8.
      * outproj(b-1) is emitted interleaved after each attention span of
        batch b, so the out-of-order Tile scheduler uses its matmuls to
        fill PE slack in the EXP-paced attention phase. proj(b+1) fills
        whatever is left (it is emitted later = lower priority).
  - The v1 softmax epilogue (DVE reciprocal 3.3us + den DMA + ao-mul all
    serialized on the scalar queue) stalled the exp stream ~5us/span-pair.
    Now: reciprocal_approx_fast (DVE, ~0.7us, 18-bit - plenty for bf16
    outputs), den row-1 hop + broadcasts + acc evictions on gpsimd, ao
    muls + output DMAs on the vector queue. The scalar queue runs exps
    back-to-back only.
  - Wo is no longer SBUF-resident (64KB/partition): it streams per 512-col
    slice during the attention window (DMA is idle there). Frees SBUF for
    x-block triple buffering.
"""

import numpy as np
import ml_dtypes

import concourse.bacc as bacc
import concourse.mybir as mybir
import concourse.tile as tile
import concourse.bass_utils as bass_utils
from concourse.masks import make_identity

B, S, E, H = 4, 2048, 2048, 16
HD = E // H            # 128
N_CORES = 8
H_LOC = H // N_CORES   # 2 heads per core
F_LOC = H_LOC * HD     # 256 features per core (head slice)
S_LOC = S // N_CORES   # 256 sequence rows per core (out-proj slice)
P = 128
NS = 512               # matmul free-dim span
EC = E // P            # 16 contraction chunks
QSP = S // NS          # 4 q-spans per (b, h)
KCH = S // P           # 16 k-chunks
NBLK = S // NS         # 4 proj blocks per tensor
NF = E // NS           # 4 out-proj feature spans
INV_SQRT_HD = float(1.0 / np.sqrt(HD))

BF16 = mybir.dt.bfloat16
F32 = mybir.dt.float32

_cached_nc = None


def _build():
    nc = bacc.Bacc("TRN2", target_bir_lowering=False, debug=False,
                   num_devices=N_CORES)

    # ---------------- I/O ----------------
    qt_d = nc.dram_tensor("qt", [B, E, S], BF16, kind="ExternalInput")
    kt_d = nc.dram_tensor("kt", [B, E, S], BF16, kind="ExternalInput")
    vt_d = nc.dram_tensor("vt", [B, E, S], BF16, kind="ExternalInput")
    wqt_d = nc.dram_tensor("wqt", [E, F_LOC], BF16, kind="ExternalInput")
    wkt_d = nc.dram_tensor("wkt", [E, F_LOC], BF16, kind="ExternalInput")
    wvt_d = nc.dram_tensor("wvt", [E, F_LOC], BF16, kind="ExternalInput")
    wot_d = nc.dram_tensor("wot", [E, E], BF16, kind="ExternalInput")
    bias_d = nc.dram_tensor("bias_bc", [P, E], BF16, kind="ExternalInput")
    masks_d = nc.dram_tensor("masks", [4, P, NS], BF16, kind="ExternalInput")
    out_d = nc.dram_tensor("out", [B, S_LOC, E], F32, kind="ExternalOutput")

    wot_v = wot_d.ap().rearrange("(ec p) f -> p ec f", p=P)

    with tile.TileContext(nc) as tc:
        with (
            tc.tile_pool(name="wconst", bufs=1) as wconst,
            tc.tile_pool(name="proj", bufs=2) as proj,
            tc.tile_pool(name="xs", bufs=3) as xs,
            tc.tile_pool(name="wop", bufs=3) as wop,
            tc.tile_pool(name="lhsp", bufs=2) as lhsp,
            tc.tile_pool(name="expp", bufs=8) as expp,
            tc.tile_pool(name="smallp", bufs=2) as smallp,
            tc.tile_pool(name="outp", bufs=2) as outp,
            tc.tile_pool(name="ps_s", bufs=2, space="PSUM") as ps_s,
            tc.tile_pool(name="ps_acc", bufs=2, space="PSUM") as ps_acc,
            tc.tile_pool(name="ps_den", bufs=1, space="PSUM") as ps_den,
            tc.tile_pool(name="ps_po", bufs=3, space="PSUM") as ps_po,
            tc.tile_pool(name="dram", bufs=1, space="DRAM") as dram,
        ):
            # ------------ constants / weights resident in SBUF ------------
            # wq + the first x block land first (the startup critical path);
            # wk/wv dmas are emitted just before their first use inside the
            # b==0 projection loop.
            wq_sb = wconst.tile([P, EC, F_LOC], BF16, tag="wq")
            wk_sb = wconst.tile([P, EC, F_LOC], BF16, tag="wk")
            wv_sb = wconst.tile([P, EC, F_LOC], BF16, tag="wv")
            nc.sync.dma_start(wq_sb[:], wqt_d.ap().rearrange("(ec p) f -> p ec f", p=P))
            bias_sb = wconst.tile([P, E], BF16, tag="bias")
            nc.scalar.dma_start(bias_sb[:], bias_d.ap())
            mask_sb = wconst.tile([P, 4, NS], BF16, tag="mask")
            nc.scalar.dma_start(mask_sb[:], masks_d.ap().rearrange("r p q -> p r q"))
            # all-ones [P, 1] lhsT: folds the 128 k-partitions of a dacc tile
            # into a single psum row (the per-head softmax denominator).
            ones_sb = wconst.tile([P, 1], BF16, tag="ones")
            nc.vector.memset(ones_sb[:], 1.0)
            ident_sb = wconst.tile([P, P], BF16, tag="ident")
            make_identity(nc, ident_sb[:])

            a2a_in = [dram.tile([N_CORES, F_LOC, S_LOC], BF16,
                                tag=f"a2a_in{b}", name=f"a2a_in{b}")
                      for b in range(B)]
            a2a_out = [dram.tile([N_CORES, F_LOC, S_LOC], BF16,
                                 tag=f"a2a_out{b}", name=f"a2a_out{b}")
                       for b in range(B)]

            # out-proj lhs tiles, keyed by batch (loaded lazily at nf==0);
            # pending (psum, b, nf, sc) evictions, flushed one span later so
            # the DVE never queue-blocks on in-flight filler matmuls.
            lhs_tiles = {}
            pending_evict = []

            def flush_evicts():
                while pending_evict:
                    ps, eb, enf, esc = pending_evict.pop(0)
                    o_t = outp.tile([P, NS], F32, tag="o", name="o_t")
                    nc.vector.tensor_add(o_t[:], ps[:],
                                         bias_sb[:, enf * NS:(enf + 1) * NS])
                    nc.gpsimd.dma_start(
                        out_d.ap()[eb, esc * P:(esc + 1) * P,
                                   enf * NS:(enf + 1) * NS],
                        o_t[:])

            def outproj_nf(b, nf):
                """One 512-wide feature span of batch b's out-projection.

                Emitted interleaved between attention spans of batch b+1 so
                its matmuls fill PE slack there. Evictions are deferred to
                the next call (the matmuls have completed by then).
                """
                flush_evicts()
                if nf == 0:
                    lts = []
                    for sc in range(S_LOC // P):
                        l_t = lhsp.tile([P, EC, P], BF16, tag="lo", name="lo_t")
                        nc.gpsimd.dma_start(
                            l_t[:],
                            a2a_out[b][:, :, sc * P:(sc + 1) * P]
                            .rearrange("r (dc p) s -> p (r dc) s", p=P))
                        lts.append(l_t)
                    lhs_tiles[b] = lts
                wo_t = wop.tile([P, EC, NS], BF16, tag="wo")
                nc.sync.dma_start(wo_t[:], wot_v[:, :, nf * NS:(nf + 1) * NS])
                for sc in range(S_LOC // P):
                    ps = ps_po.tile([P, NS], F32, tag="po", name="ops")
                    for ec in range(EC):
                        nc.tensor.matmul(ps[:], lhs_tiles[b][sc][:, ec, :],
                                         wo_t[:, ec, :],
                                         start=(ec == 0), stop=(ec == EC - 1))
                    pending_evict.append((ps, b, nf, sc))

            for b in range(B):
                # -------- Q/K/V projections, all in T-layout [d, s] -------
                # One mega-DMA [128, EC, 512] per 512-col block; per head a
                # single psum bank accumulates serially over all 16 e-chunks.
                qT_sb = proj.tile([P, H_LOC, S], BF16, tag="qT")
                kT_sb = proj.tile([P, H_LOC, S], BF16, tag="kT")
                vT_sb = proj.tile([P, H_LOC, S], BF16, tag="vT", bufs=1)
                v_sb = proj.tile([P, KCH, F_LOC], BF16, tag="v", bufs=1)

                for src_d, w_sb, dst, wsrc_d in (
                        (qt_d, wq_sb, qT_sb, None),
                        (kt_d, wk_sb, kT_sb, wkt_d),
                        (vt_d, wv_sb, vT_sb, wvt_d)):
                    if b == 0 and wsrc_d is not None:
                        nc.sync.dma_start(
                            w_sb[:],
                            wsrc_d.ap().rearrange("(ec p) f -> p ec f", p=P))
                    src_v = src_d.ap()[b].rearrange("(ec p) s -> p ec s", p=P)
                    for blk in range(NBLK):
                        x_t = xs.tile([P, EC, NS], BF16, tag="x")
                        nc.sync.dma_start(x_t[:],
                                          src_v[:, :, blk * NS:(blk + 1) * NS])
                        for h in range(H_LOC):
                            ps = ps_po.tile([P, NS], F32, tag="po", name="pps")
                            for ec in range(EC):
                                nc.tensor.matmul(
                                    ps[:], w_sb[:, ec, h * HD:(h + 1) * HD],
                                    x_t[:, ec, :],
                                    start=(ec == 0), stop=(ec == EC - 1))
                            nc.vector.tensor_copy(
                                dst[:, h, blk * NS:(blk + 1) * NS], ps[:])

                # v [s, d] from vT via PE transposes
                for sc in range(KCH):
                    for h in range(H_LOC):
                        tps = ps_po.tile([P, P], BF16, tag="po", name="tps")
                        nc.tensor.transpose(tps[:], vT_sb[:, h, sc * P:(sc + 1) * P],
                                            ident_sb[:])
                        nc.vector.tensor_copy(v_sb[:, sc, h * HD:(h + 1) * HD],
                                              tps[:])

                # ----- attention: q-span outer, head inner; the two heads'
                # denominators pack into one [2, NS] psum via one-hot lhsT ----
                for i in range(QSP):
                    n_k = 4 * i + 4
                    for h in range(H_LOC):
                        outT_ps = ps_acc.tile([P, NS], F32, tag="acc",
                                              name=f"acc{h}")
                        dacc = expp.tile([P, NS], BF16, tag="dacc", bufs=2)
                        for j in range(n_k):
                            s_ps = ps_s.tile([P, NS], F32, tag="s")
                            nc.tensor.matmul(
                                s_ps[:], kT_sb[:, h, j * P:(j + 1) * P],
                                qT_sb[:, h, i * NS:(i + 1) * NS],
                                start=True, stop=True)
                            e_t = expp.tile([P, NS], BF16, tag="e", bufs=7)
                            nc.scalar.activation(e_t[:], s_ps[:],
                                                 mybir.ActivationFunctionType.Exp,
                                                 scale=INV_SQRT_HD)
                            r = j - 4 * i
                            if r >= 0:
                                nc.vector.tensor_mul(e_t[:], e_t[:], mask_sb[:, r, :])
                            # denominator partials accumulate on DVE (bf16)
                            if j == 0:
                                nc.vector.tensor_copy(dacc[:], e_t[:])
                            else:
                                nc.vector.tensor_add(dacc[:], dacc[:], e_t[:])
                            nc.tensor.matmul(outT_ps[:], v_sb[:, j, h * HD:(h + 1) * HD],
                                             e_t[:], start=(j == 0), stop=(j == n_k - 1))
                        # per-head epilogue, fully independent of the other
                        # head and free of any DMA hop (a tiny SBUF->SBUF dma
                        # here would queue behind megabytes of bulk x/wo
                        # descriptors): fold dacc -> [1,NS] den psum row,
                        # approx-reciprocal, broadcast, normalize, ship out.
                        den_ps = ps_den.tile([1, NS], F32, tag="den")
                        nc.tensor.matmul(den_ps[:], ones_sb[:], dacc[:],
                                         start=True, stop=True)
                        aof = smallp.tile([P, NS], BF16, tag="aof", bufs=2,
                                          name="aof")
                        nc.vector.tensor_copy(aof[:], outT_ps[:])
                        den_rec = smallp.tile([1, NS], F32, tag="den_rec", bufs=1)
                        nc.vector.reciprocal_approx_fast(den_rec[:], den_ps[:])
                        den_bc = smallp.tile([P, NS], F32, tag="den_bc")
                        nc.gpsimd.partition_broadcast(den_bc[:], den_rec[:])
                        ao = smallp.tile([P, NS], BF16, tag="ao")
                        nc.vector.tensor_mul(ao[:], aof[:], den_bc[:])
                        dst = a2a_in[b][2 * i:2 * i + 2, h * HD:(h + 1) * HD, :]
                        nc.sync.dma_start(dst.transpose([1, 0, 2]),
                                          ao[:].rearrange("p (g q) -> p g q", g=2))
                    # out-projection of the PREVIOUS batch, one feature span
                    # per attention span: ready PE filler for the exp-paced
                    # attention window, at lower scheduler priority.
                    if b > 0:
                        outproj_nf(b - 1, i)

                # ---------------- head -> sequence redistribution ---------
                nc.gpsimd.collective_compute(
                    "AllToAll", mybir.AluOpType.bypass,
                    replica_groups=[list(range(N_CORES))],
                    ins=[a2a_in[b][:].opt()], outs=[a2a_out[b][:].opt()])

            for nf in range(NF):
                outproj_nf(B - 1, nf)
            flush_evicts()

    nc.compile()
    return nc


def _get_nc():
    global _cached_nc
    if _cached_nc is None:
        _cached_nc = _build()
    return _cached_nc


def kernel(query, key, value, key_padding_mask, Wq, Wk, Wv, Wo, bo):
    query = np.asarray(query, dtype=np.float32)
    key = np.asarray(key, dtype=np.float32)
    value = np.asarray(value, dtype=np.float32)
    Wq = np.asarray(Wq, dtype=np.float32)
    Wk = np.asarray(Wk, dtype=np.float32)
    Wv = np.asarray(Wv, dtype=np.float32)
    Wo = np.asarray(Wo, dtype=np.float32)
    bo = np.asarray(bo, dtype=np.float32)

    bf = ml_dtypes.bfloat16
    # host-side layout prep: transpose activations to [b, e, s], cast to bf16
    qt = np.ascontiguousarray(query.transpose(0, 2, 1)).astype(bf)
    kt = np.ascontiguousarray(key.transpose(0, 2, 1)).astype(bf)
    vt = np.ascontiguousarray(value.transpose(0, 2, 1)).astype(bf)
    wot = np.ascontiguousarray(Wo.T).astype(bf)
    bias_bc = np.broadcast_to(bo, (P, E)).astype(bf)

    # causal masks for the 4 diagonal shifts: mask_r[kk, qq] = kk <= qq - 128 r
    kk = np.arange(P)[:, None]
    qq = np.arange(NS)[None, :]
    masks = np.stack([(kk <= qq - P * r) for r in range(4)]).astype(bf)

    in_maps = []
    for c in range(N_CORES):
        sl = slice(c * F_LOC, (c + 1) * F_LOC)
        in_maps.append(dict(
            qt=qt, kt=kt, vt=vt,
            wqt=np.ascontiguousarray(Wq[sl].T).astype(bf),
            wkt=np.ascontiguousarray(Wk[sl].T).astype(bf),
            wvt=np.ascontiguousarray(Wv[sl].T).astype(bf),
            wot=wot, bias_bc=bias_bc, masks=masks,
        ))

    nc = _get_nc()
    res = bass_utils.run_bass_kernel_spmd(
        nc, in_maps, core_ids=list(range(N_CORES)), trace=False)

    out = np.empty((B, S, E), dtype=np.float32)
    for c in range(N_CORES):
        out[:, c * S_LOC:(c + 1) * S_LOC, :] = res.results[c]["out"]
    return out


# revision 16
# speedup vs baseline: 1.4148x; 1.0497x over previous
"""Causal multi-head attention on 8 Trainium2 NeuronCores.

Problem: B=4, S=2048, E=2048, H=16 heads (HD=128), fp32 I/O.

Sharding (tensor-parallel on heads + sequence-parallel out-proj):
  - Every core holds the full (host-transposed, bf16-cast) activations and
    projects Q/K/V only for its 2 heads (per-core slices of Wq/Wk/Wv rows).
  - Attention (scores -> exp -> normalize -> @V) runs fully local per
    (batch, head), producing attn_outT [d_local=256, s=2048] per batch.
  - An AllToAll redistributes attn_outT from head-sharded to
    sequence-sharded: core c ends with attn_outT [e=2048, s_c=256] per batch.
  - Out-projection is computed for the core's 256 sequence rows per batch;
    the host concatenates row-slices - no further reduction needed.

v2 design notes (from perfetto analysis of v1 @ 1011us):
  - The PE only reaches its full 2.4GHz clock after ~3us of continuous
    execution; every idle gap drops it to 1.2GHz. So the whole kernel is
    organized to keep the PE stream dense:
      * proj uses one mega-DMA [128,16,512] per 512-col block (sync queue,
        ~12 issues/batch instead of ~96) and 2 serially-accumulated psum
        banks, leaving banks for overlap.
      * psum pools: scores 2 + attn-acc 2 + den 1 + proj/outproj 3 = 8.
      * outproj(b-1) is emitted interleaved after each attention span of
        batch b, so the out-of-order Tile scheduler uses its matmuls to
        fill PE slack in the EXP-paced attention phase. proj(b+1) fills
        whatever is left (it is emitted later = lower priority).
  - The v1 softmax epilogue (DVE reciprocal 3.3us + den DMA + ao-mul all
    serialized on the scalar queue) stalled the exp stream ~5us/span-pair.
    Now: reciprocal_approx_fast (DVE, ~0.7us, 18-bit - plenty for bf16
    outputs), den row-1 hop + broadcasts + acc evictions on gpsimd, ao
    muls + output DMAs on the vector queue. The scalar queue runs exps
    back-to-back only.
  - Wo is no longer SBUF-resident (64KB/partition): it streams per 512-col
    slice during the attention window (DMA is idle there). Frees SBUF for
    x-block triple buffering.
"""

import numpy as np
import ml_dtypes

import concourse.bacc as bacc
import concourse.mybir as mybir
import concourse.tile as tile
import concourse.bass_utils as bass_utils
from concourse.masks import make_identity

B, S, E, H = 4, 2048, 2048, 16
HD = E // H            # 128
N_CORES = 8
H_LOC = H // N_CORES   # 2 heads per core
F_LOC = H_LOC * HD     # 256 features per core (head slice)
S_LOC = S // N_CORES   # 256 sequence rows per core (out-proj slice)
P = 128
NS = 512               # matmul free-dim span
EC = E // P            # 16 contraction chunks
QSP = S // NS          # 4 q-spans per (b, h)
KCH = S // P           # 16 k-chunks
NBLK = S // NS         # 4 proj blocks per tensor
NF = E // NS           # 4 out-proj feature spans
INV_SQRT_HD = float(1.0 / np.sqrt(HD))

BF16 = mybir.dt.bfloat16
F32 = mybir.dt.float32

_cached_nc = None


def _build():
    nc = bacc.Bacc("TRN2", target_bir_lowering=False, debug=False,
                   num_devices=N_CORES)

    # ---------------- I/O ----------------
    qt_d = nc.dram_tensor("qt", [B, E, S], BF16, kind="ExternalInput")
    kt_d = nc.dram_tensor("kt", [B, E, S], BF16, kind="ExternalInput")
    vt_d = nc.dram_tensor("vt", [B, E, S], BF16, kind="ExternalInput")
    wqt_d = nc.dram_tensor("wqt", [E, F_LOC], BF16, kind="ExternalInput")
    wkt_d = nc.dram_tensor("wkt", [E, F_LOC], BF16, kind="ExternalInput")
    wvt_d = nc.dram_tensor("wvt", [E, F_LOC], BF16, kind="ExternalInput")
    wot_d = nc.dram_tensor("wot", [E, E], BF16, kind="ExternalInput")
    bias_d = nc.dram_tensor("bias_bc", [P, E], BF16, kind="ExternalInput")
    masks_d = nc.dram_tensor("masks", [4, P, NS], BF16, kind="ExternalInput")
    out_d = nc.dram_tensor("out", [B, S_LOC, E], F32, kind="ExternalOutput")

    wot_v = wot_d.ap().rearrange("(ec p) f -> p ec f", p=P)

    with tile.TileContext(nc) as tc:
        with (
            tc.tile_pool(name="wconst", bufs=1) as wconst,
            tc.tile_pool(name="proj", bufs=2) as proj,
            tc.tile_pool(name="xs", bufs=3) as xs,
            tc.tile_pool(name="wop", bufs=3) as wop,
            tc.tile_pool(name="lhsp", bufs=2) as lhsp,
            tc.tile_pool(name="expp", bufs=8) as expp,
            tc.tile_pool(name="smallp", bufs=2) as smallp,
            tc.tile_pool(name="outp", bufs=2) as outp,
            tc.tile_pool(name="ps_s", bufs=2, space="PSUM") as ps_s,
            tc.tile_pool(name="ps_acc", bufs=2, space="PSUM") as ps_acc,
            tc.tile_pool(name="ps_den", bufs=1, space="PSUM") as ps_den,
            tc.tile_pool(name="ps_po", bufs=3, space="PSUM") as ps_po,
            tc.tile_pool(name="dram", bufs=1, space="DRAM") as dram,
        ):
            # ------------ constants / weights resident in SBUF ------------
            # wq + the first x block land first (the startup critical path);
            # wk/wv dmas are emitted just before their first use inside the
            # b==0 projection loop.
            wq_sb = wconst.tile([P, EC, F_LOC], BF16, tag="wq")
            wk_sb = wconst.tile([P, EC, F_LOC], BF16, tag="wk")
            wv_sb = wconst.tile([P, EC, F_LOC], BF16, tag="wv")
            nc.sync.dma_start(wq_sb[:], wqt_d.ap().rearrange("(ec p) f -> p ec f", p=P))
            bias_sb = wconst.tile([P, E], BF16, tag="bias")
            nc.scalar.dma_start(bias_sb[:], bias_d.ap())
            mask_sb = wconst.tile([P, 4, NS], BF16, tag="mask")
            nc.scalar.dma_start(mask_sb[:], masks_d.ap().rearrange("r p q -> p r q"))
            # all-ones [P, 1] lhsT: folds the 128 k-partitions of a dacc tile
            # into a single psum row (the per-head softmax denominator).
            ones_sb = wconst.tile([P, 1], BF16, tag="ones")
            nc.vector.memset(ones_sb[:], 1.0)
            ident_sb = wconst.tile([P, P], BF16, tag="ident")
            make_identity(nc, ident_sb[:])

            a2a_in = [dram.tile([N_CORES, F_LOC, S_LOC], BF16,
                                tag=f"a2a_in{b}", name=f"a2a_in{b}")
                      for b in range(B)]
            a2a_out = [dram.tile([N_CORES, F_LOC, S_LOC], BF16,
                                 tag=f"a2a_out{b}", name=f"a2a_out{b}")
                       for b in range(B)]

            # out-proj lhs tiles, keyed by batch (loaded lazily at nf==0);
            # pending (psum, b, nf, sc) evictions, flushed one span later so
            # the DVE never queue-blocks on in-flight filler matmuls.
            lhs_tiles = {}
            pending_evict = []

            def flush_evicts():
                while pending_evict:
                    ps, eb, enf, esc = pending_evict.pop(0)
                    o_t = outp.tile([P, NS], F32, tag="o", name="o_t")
                    nc.vector.tensor_add(o_t[:], ps[:],
                                         bias_sb[:, enf * NS:(enf + 1) * NS])
                    nc.gpsimd.dma_start(
                        out_d.ap()[eb, esc * P:(esc + 1) * P,
                                   enf * NS:(enf + 1) * NS],
                        o_t[:])

            def outproj_nf(b, nf):
                """One 512-wide feature span of batch b's out-projection.

                Emitted interleaved between attention spans of batch b+1 so
                its matmuls fill PE slack there. Evictions are deferred to
                the next call (the matmuls have completed by then).
                """
                flush_evicts()
                if nf == 0:
                    lts = []
                    for sc in range(S_LOC // P):
                        l_t = lhsp.tile([P, EC, P], BF16, tag="lo", name="lo_t")
                        nc.gpsimd.dma_start(
                            l_t[:],
                            a2a_out[b][:, :, sc * P:(sc + 1) * P]
                            .rearrange("r (dc p) s -> p (r dc) s", p=P))
                        lts.append(l_t)
                    lhs_tiles[b] = lts
                wo_t = wop.tile([P, EC, NS], BF16, tag="wo")
                nc.sync.dma_start(wo_t[:], wot_v[:, :, nf * NS:(nf + 1) * NS])
                for sc in range(S_LOC // P):
                    ps = ps_po.tile([P, NS], F32, tag="po", name="ops")
                    for ec in range(EC):
                        nc.tensor.matmul(ps[:], lhs_tiles[b][sc][:, ec, :],
                                         wo_t[:, ec, :],
                                         start=(ec == 0), stop=(ec == EC - 1))
                    pending_evict.append((ps, b, nf, sc))

            for b in range(B):
                # -------- Q/K/V projections, all in T-layout [d, s] -------
                # One mega-DMA [128, EC, 512] per 512-col block; per head a
                # single psum bank accumulates serially over all 16 e-chunks.
                qT_sb = proj.tile([P, H_LOC, S], BF16, tag="qT")
                kT_sb = proj.tile([P, H_LOC, S], BF16, tag="kT")
                vT_sb = proj.tile([P, H_LOC, S], BF16, tag="vT", bufs=1)
                v_sb = proj.tile([P, KCH, F_LOC], BF16, tag="v", bufs=1)

                for src_d, w_sb, dst, wsrc_d in (
                        (qt_d, wq_sb, qT_sb, None),
                        (kt_d, wk_sb, kT_sb, wkt_d),
                        (vt_d, wv_sb, vT_sb, wvt_d)):
                    if b == 0 and wsrc_d is not None:
                        nc.sync.dma_start(
                            w_sb[:],
                            wsrc_d.ap().rearrange("(ec p) f -> p ec f", p=P))
                    src_v = src_d.ap()[b].rearrange("(ec p) s -> p ec s", p=P)
                    for blk in range(NBLK):
                        x_t = xs.tile([P, EC, NS], BF16, tag="x")
                        nc.sync.dma_start(x_t[:],
                                          src_v[:, :, blk * NS:(blk + 1) * NS])
                        for h in range(H_LOC):
                            ps = ps_po.tile([P, NS], F32, tag="po", name="pps")
                            for ec in range(EC):
                                nc.tensor.matmul(
                                    ps[:], w_sb[:, ec, h * HD:(h + 1) * HD],
                                    x_t[:, ec, :],
                                    start=(ec == 0), stop=(ec == EC - 1))
                            nc.vector.tensor_copy(
                                dst[:, h, blk * NS:(blk + 1) * NS], ps[:])

                # v [s, d] from vT via PE transposes
                for sc in range(KCH):
                    for h in range(H_LOC):
                        tps = ps_po.tile([P, P], BF16, tag="po", name="tps")
                        nc.tensor.transpose(tps[:], vT_sb[:, h, sc * P:(sc + 1) * P],
                                            ident_sb[:])
                        nc.vector.tensor_copy(v_sb[:, sc, h * HD:(h + 1) * HD],
                                              tps[:])

                # ----- attention: q-span outer, head inner; the two heads'
                # denominators pack into one [2, NS] psum via one-hot lhsT ----
                for i in range(QSP):
                    n_k = 4 * i + 4
                    for h in range(H_LOC):
                        outT_ps = ps_acc.tile([P, NS], F32, tag="acc",
                                              name=f"acc{h}")
                        dacc = expp.tile([P, NS], BF16, tag="dacc", bufs=2)
                        for j in range(n_k):
                            s_ps = ps_s.tile([P, NS], F32, tag="s")
                            nc.tensor.matmul(
                                s_ps[:], kT_sb[:, h, j * P:(j + 1) * P],
                                qT_sb[:, h, i * NS:(i + 1) * NS],
                                start=True, stop=True)
                            e_t = expp.tile([P, NS], BF16, tag="e", bufs=7)
                            nc.scalar.activation(e_t[:], s_ps[:],
                                                 mybir.ActivationFunctionType.Exp,
                                                 scale=INV_SQRT_HD)
                            r = j - 4 * i
                            if r >= 0:
                                nc.vector.tensor_mul(e_t[:], e_t[:], mask_sb[:, r, :])
                            # denominator partials accumulate on DVE (bf16)
                            if j == 0:
                                nc.vector.tensor_copy(dacc[:], e_t[:])
                            else:
                                nc.vector.tensor_add(dacc[:], dacc[:], e_t[:])
                            nc.tensor.matmul(outT_ps[:], v_sb[:, j, h * HD:(h + 1) * HD],
                                             e_t[:], start=(j == 0), stop=(j == n_k - 1))
                        # per-head epilogue, fully independent of the other
                        # head and free of any DMA hop (a tiny SBUF->SBUF dma
                        # here would queue behind megabytes of bulk x/wo
                        # descriptors): fold dacc -> [1,NS] den psum row,
                        # approx-reciprocal, broadcast, normalize, ship out.
                        den_ps = ps_den.tile([1, NS], F32, tag="den")
                        nc.tensor.matmul(den_ps[:], ones_sb[:], dacc[:],
                                         start=True, stop=True)
                        aof = smallp.tile([P, NS], BF16, tag="aof", bufs=2,
                                          name="aof")
                        nc.vector.tensor_copy(aof[:], outT_ps[:])
                        den_rec = smallp.tile([1, NS], F32, tag="den_rec", bufs=1)
                        nc.vector.reciprocal_approx_fast(den_rec[:], den_ps[:])
                        den_bc = smallp.tile([P, NS], F32, tag="den_bc")
                        nc.gpsimd.partition_broadcast(den_bc[:], den_rec[:])
                        ao = smallp.tile([P, NS], BF16, tag="ao")
                        nc.vector.tensor_mul(ao[:], aof[:], den_bc[:])
                        dst = a2a_in[b][2 * i:2 * i + 2, h * HD:(h + 1) * HD, :]
                        nc.sync.dma_start(dst.transpose([1, 0, 2]),
                                          ao[:].rearrange("p (g q) -> p g q", g=2))
                    # out-projection of the PREVIOUS batch, one feature span
                    # per attention span: ready PE filler for the exp-paced
                    # attention window, at lower scheduler priority. For the
                    # last batch, outproj(B-2) is deferred to the tail so its
                    # matmuls hide the a2a(B-1) peer-skew wait (the scheduler
                    # still backfills attention(B-1) slack with them).
                    if 0 < b < B - 1:
                        outproj_nf(b - 1, i)

                # ---------------- head -> sequence redistribution ---------
                nc.gpsimd.collective_compute(
                    "AllToAll", mybir.AluOpType.bypass,
                    replica_groups=[list(range(N_CORES))],
                    ins=[a2a_in[b][:].opt()], outs=[a2a_out[b][:].opt()])

            for nf in range(NF):
                outproj_nf(B - 2, nf)
            for nf in range(NF):
                outproj_nf(B - 1, nf)
            flush_evicts()

    nc.compile()
    return nc


def _get_nc():
    global _cached_nc
    if _cached_nc is None:
        _cached_nc = _build()
    return _cached_nc


def kernel(query, key, value, key_padding_mask, Wq, Wk, Wv, Wo, bo):
    query = np.asarray(query, dtype=np.float32)
    key = np.asarray(key, dtype=np.float32)
    value = np.asarray(value, dtype=np.float32)
    Wq = np.asarray(Wq, dtype=np.float32)
    Wk = np.asarray(Wk, dtype=np.float32)
    Wv = np.asarray(Wv, dtype=np.float32)
    Wo = np.asarray(Wo, dtype=np.float32)
    bo = np.asarray(bo, dtype=np.float32)

    bf = ml_dtypes.bfloat16
    # host-side layout prep: transpose activations to [b, e, s], cast to bf16
    qt = np.ascontiguousarray(query.transpose(0, 2, 1)).astype(bf)
    kt = np.ascontiguousarray(key.transpose(0, 2, 1)).astype(bf)
    vt = np.ascontiguousarray(value.transpose(0, 2, 1)).astype(bf)
    wot = np.ascontiguousarray(Wo.T).astype(bf)
    bias_bc = np.broadcast_to(bo, (P, E)).astype(bf)

    # causal masks for the 4 diagonal shifts: mask_r[kk, qq] = kk <= qq - 128 r
    kk = np.arange(P)[:, None]
    qq = np.arange(NS)[None, :]
    masks = np.stack([(kk <= qq - P * r) for r in range(4)]).astype(bf)

    in_maps = []
    for c in range(N_CORES):
        sl = slice(c * F_LOC, (c + 1) * F_LOC)
        in_maps.append(dict(
            qt=qt, kt=kt, vt=vt,
            wqt=np.ascontiguousarray(Wq[sl].T).astype(bf),
            wkt=np.ascontiguousarray(Wk[sl].T).astype(bf),
            wvt=np.ascontiguousarray(Wv[sl].T).astype(bf),
            wot=wot, bias_bc=bias_bc, masks=masks,
        ))

    nc = _get_nc()
    res = bass_utils.run_bass_kernel_spmd(
        nc, in_maps, core_ids=list(range(N_CORES)), trace=False)

    out = np.empty((B, S, E), dtype=np.float32)
    for c in range(N_CORES):
        out[:, c * S_LOC:(c + 1) * S_LOC, :] = res.results[c]["out"]
    return out


# revision 19
# speedup vs baseline: 1.4195x; 1.0033x over previous
"""Causal multi-head attention on 8 Trainium2 NeuronCores.

Problem: B=4, S=2048, E=2048, H=16 heads (HD=128), fp32 I/O.

Sharding (tensor-parallel on heads + sequence-parallel out-proj):
  - Every core holds the full (host-transposed, bf16-cast) activations and
    projects Q/K/V only for its 2 heads (per-core slices of Wq/Wk/Wv rows).
  - Attention (scores -> exp -> normalize -> @V) runs fully local per
    (batch, head), producing attn_outT [d_local=256, s=2048] per batch.
  - An AllToAll redistributes attn_outT from head-sharded to
    sequence-sharded: core c ends with attn_outT [e=2048, s_c=256] per batch.
  - Out-projection is computed for the core's 256 sequence rows per batch;
    the host concatenates row-slices - no further reduction needed.

v2 design notes (from perfetto analysis of v1 @ 1011us):
  - The PE only reaches its full 2.4GHz clock after ~3us of continuous
    execution; every idle gap drops it to 1.2GHz. So the whole kernel is
    organized to keep the PE stream dense:
      * proj uses one mega-DMA [128,16,512] per 512-col block (sync queue,
        ~12 issues/batch instead of ~96) and 2 serially-accumulated psum
        banks, leaving banks for overlap.
      * psum pools: scores 2 + attn-acc 2 + den 1 + proj/outproj 3 = 8.
      * outproj(b-1) is emitted interleaved after each attention span of
        batch b, so the out-of-order Tile scheduler uses its matmuls to
        fill PE slack in the EXP-paced attention phase. proj(b+1) fills
        whatever is left (it is emitted later = lower priority).
  - The v1 softmax epilogue (DVE reciprocal 3.3us + den DMA + ao-mul all
    serialized on the scalar queue) stalled the exp stream ~5us/span-pair.
    Now: reciprocal_approx_fast (DVE, ~0.7us, 18-bit - plenty for bf16
    outputs), den row-1 hop + broadcasts + acc evictions on gpsimd, ao
    muls + output DMAs on the vector queue. The scalar queue runs exps
    back-to-back only.
  - Wo is no longer SBUF-resident (64KB/partition): it streams per 512-col
    slice during the attention window (DMA is idle there). Frees SBUF for
    x-block triple buffering.
"""

import numpy as np
import ml_dtypes

import concourse.bacc as bacc
import concourse.mybir as mybir
import concourse.tile as tile
import concourse.bass_utils as bass_utils
from concourse.masks import make_identity

B, S, E, H = 4, 2048, 2048, 16
HD = E // H            # 128
N_CORES = 8
H_LOC = H // N_CORES   # 2 heads per core
F_LOC = H_LOC * HD     # 256 features per core (head slice)
S_LOC = S // N_CORES   # 256 sequence rows per core (out-proj slice)
P = 128
NS = 512               # matmul free-dim span
EC = E // P            # 16 contraction chunks
QSP = S // NS          # 4 q-spans per (b, h)
KCH = S // P           # 16 k-chunks
NBLK = S // NS         # 4 proj blocks per tensor
NF = E // NS           # 4 out-proj feature spans
INV_SQRT_HD = float(1.0 / np.sqrt(HD))

BF16 = mybir.dt.bfloat16
F32 = mybir.dt.float32

_cached_nc = None


def _build():
    nc = bacc.Bacc("TRN2", target_bir_lowering=False, debug=False,
                   num_devices=N_CORES)

    # ---------------- I/O ----------------
    # all inputs are host-pre-arranged so every DMA reads contiguous
    # per-partition runs (x: 16KB, weights: 8KB, wo: 8KB) - 2KB+ descriptors
    # keep the 16 DMA engines at full rate.
    qt_d = nc.dram_tensor("qt", [B, NBLK, P, EC, NS], BF16, kind="ExternalInput")
    kt_d = nc.dram_tensor("kt", [B, NBLK, P, EC, NS], BF16, kind="ExternalInput")
    vt_d = nc.dram_tensor("vt", [B, NBLK, P, EC, NS], BF16, kind="ExternalInput")
    wqt_d = nc.dram_tensor("wqt", [P, EC, F_LOC], BF16, kind="ExternalInput")
    wkt_d = nc.dram_tensor("wkt", [P, EC, F_LOC], BF16, kind="ExternalInput")
    wvt_d = nc.dram_tensor("wvt", [P, EC, F_LOC], BF16, kind="ExternalInput")
    wot_d = nc.dram_tensor("wot", [NF, P, EC, NS], BF16, kind="ExternalInput")
    bias_d = nc.dram_tensor("bias_bc", [P, E], BF16, kind="ExternalInput")
    masks_d = nc.dram_tensor("masks", [P, 4, NS], BF16, kind="ExternalInput")
    out_d = nc.dram_tensor("out", [B, S_LOC, E], F32, kind="ExternalOutput")

    with tile.TileContext(nc) as tc:
        with (
            tc.tile_pool(name="wconst", bufs=1) as wconst,
            tc.tile_pool(name="proj", bufs=2) as proj,
            tc.tile_pool(name="xs", bufs=3) as xs,
            tc.tile_pool(name="wop", bufs=3) as wop,
            tc.tile_pool(name="lhsp", bufs=2) as lhsp,
            tc.tile_pool(name="expp", bufs=8) as expp,
            tc.tile_pool(name="smallp", bufs=2) as smallp,
            tc.tile_pool(name="outp", bufs=2) as outp,
            tc.tile_pool(name="ps_s", bufs=2, space="PSUM") as ps_s,
            tc.tile_pool(name="ps_acc", bufs=2, space="PSUM") as ps_acc,
            tc.tile_pool(name="ps_den", bufs=1, space="PSUM") as ps_den,
            tc.tile_pool(name="ps_po", bufs=3, space="PSUM") as ps_po,
            tc.tile_pool(name="dram", bufs=1, space="DRAM") as dram,
        ):
            # ------------ constants / weights resident in SBUF ------------
            # wq + the first x block land first (the startup critical path);
            # wk/wv dmas are emitted just before their first use inside the
            # b==0 projection loop.
            wq_sb = wconst.tile([P, EC, F_LOC], BF16, tag="wq")
            wk_sb = wconst.tile([P, EC, F_LOC], BF16, tag="wk")
            wv_sb = wconst.tile([P, EC, F_LOC], BF16, tag="wv")
            nc.sync.dma_start(wq_sb[:], wqt_d.ap())
            bias_sb = wconst.tile([P, E], BF16, tag="bias")
            nc.scalar.dma_start(bias_sb[:], bias_d.ap())
            mask_sb = wconst.tile([P, 4, NS], BF16, tag="mask")
            nc.scalar.dma_start(mask_sb[:], masks_d.ap())
            # all-ones [P, 1] lhsT: folds the 128 k-partitions of a dacc tile
            # into a single psum row (the per-head softmax denominator).
            ones_sb = wconst.tile([P, 1], BF16, tag="ones")
            nc.vector.memset(ones_sb[:], 1.0)
            ident_sb = wconst.tile([P, P], BF16, tag="ident")
            make_identity(nc, ident_sb[:])

            a2a_in = [dram.tile([N_CORES, P, H_LOC, S_LOC], BF16,
                                tag=f"a2a_in{b}", name=f"a2a_in{b}")
                      for b in range(B)]
            a2a_out = [dram.tile([N_CORES, P, H_LOC, S_LOC], BF16,
                                 tag=f"a2a_out{b}", name=f"a2a_out{b}")
                       for b in range(B)]

            # out-proj lhs tiles, keyed by batch (loaded lazily at nf==0);
            # pending (psum, b, nf, sc) evictions, flushed one span later so
            # the DVE never queue-blocks on in-flight filler matmuls.
            lhs_tiles = {}
            pending_evict = []

            def flush_evicts():
                while pending_evict:
                    ps, eb, enf, esc = pending_evict.pop(0)
                    o_t = outp.tile([P, NS], F32, tag="o", name="o_t")
                    nc.vector.tensor_add(o_t[:], ps[:],
                                         bias_sb[:, enf * NS:(enf + 1) * NS])
                    nc.gpsimd.dma_start(
                        out_d.ap()[eb, esc * P:(esc + 1) * P,
                                   enf * NS:(enf + 1) * NS],
                        o_t[:])

            def outproj_nf(b, nf):
                """One 512-wide feature span of batch b's out-projection.

                Emitted interleaved between attention spans of batch b+1 so
                its matmuls fill PE slack there. Evictions are deferred to
                the next call (the matmuls have completed by then).
                """
                flush_evicts()
                if nf == 0:
                    l_t = lhsp.tile([P, N_CORES, H_LOC, S_LOC], BF16,
                                    tag="lo", bufs=1, name="lo_t")
                    nc.gpsimd.dma_start(
                        l_t[:],
                        a2a_out[b][:].rearrange("r p dc s -> p r dc s"))
                    lhs_tiles[b] = l_t
                wo_t = wop.tile([P, EC, NS], BF16, tag="wo")
                nc.sync.dma_start(wo_t[:], wot_d.ap()[nf])
                for sc in range(S_LOC // P):
                    ps = ps_po.tile([P, NS], F32, tag="po", name="ops")
                    for ec in range(EC):
                        nc.tensor.matmul(
                            ps[:],
                            lhs_tiles[b][:, ec // H_LOC, ec % H_LOC,
                                         sc * P:(sc + 1) * P],
                            wo_t[:, ec, :],
                            start=(ec == 0), stop=(ec == EC - 1))
                    pending_evict.append((ps, b, nf, sc))

            for b in range(B):
                # -------- Q/K/V projections, all in T-layout [d, s] -------
                # One mega-DMA [128, EC, 512] per 512-col block; per head a
                # single psum bank accumulates serially over all 16 e-chunks.
                qT_sb = proj.tile([P, H_LOC, S], BF16, tag="qT")
                kT_sb = proj.tile([P, H_LOC, S], BF16, tag="kT")
                vT_sb = proj.tile([P, H_LOC, S], BF16, tag="vT", bufs=1)
                v_sb = proj.tile([P, KCH, F_LOC], BF16, tag="v", bufs=1)

                for src_d, w_sb, dst, wsrc_d in (
                        (qt_d, wq_sb, qT_sb, None),
                        (kt_d, wk_sb, kT_sb, wkt_d),
                        (vt_d, wv_sb, vT_sb, wvt_d)):
                    if b == 0 and wsrc_d is not None:
                        nc.sync.dma_start(w_sb[:], wsrc_d.ap())
                    for blk in range(NBLK):
                        x_t = xs.tile([P, EC, NS], BF16, tag="x")
                        nc.sync.dma_start(x_t[:], src_d.ap()[b, blk])
                        for h in range(H_LOC):
                            ps = ps_po.tile([P, NS], F32, tag="po", name="pps")
                            for ec in range(EC):
                                nc.tensor.matmul(
                                    ps[:], w_sb[:, ec, h * HD:(h + 1) * HD],
                                    x_t[:, ec, :],
                                    start=(ec == 0), stop=(ec == EC - 1))
                            nc.vector.tensor_copy(
                                dst[:, h, blk * NS:(blk + 1) * NS], ps[:])

                # v [s, d] from vT via PE transposes
                for sc in range(KCH):
                    for h in range(H_LOC):
                        tps = ps_po.tile([P, P], BF16, tag="po", name="tps")
                        nc.tensor.transpose(tps[:], vT_sb[:, h, sc * P:(sc + 1) * P],
                                            ident_sb[:])
                        nc.vector.tensor_copy(v_sb[:, sc, h * HD:(h + 1) * HD],
                                              tps[:])

                # ----- attention: q-span outer, head inner; the two heads'
                # denominators pack into one [2, NS] psum via one-hot lhsT ----
                for i in range(QSP):
                    n_k = 4 * i + 4
                    for h in range(H_LOC):
                        outT_ps = ps_acc.tile([P, NS], F32, tag="acc",
                                              name=f"acc{h}")
                        dacc = expp.tile([P, NS], BF16, tag="dacc", bufs=2)
                        for j in range(n_k):
                            s_ps = ps_s.tile([P, NS], F32, tag="s")
                            nc.tensor.matmul(
                                s_ps[:], kT_sb[:, h, j * P:(j + 1) * P],
                                qT_sb[:, h, i * NS:(i + 1) * NS],
                                start=True, stop=True)
                            e_t = expp.tile([P, NS], BF16, tag="e", bufs=7)
                            nc.scalar.activation(e_t[:], s_ps[:],
                                                 mybir.ActivationFunctionType.Exp,
                                                 scale=INV_SQRT_HD)
                            r = j - 4 * i
                            if r >= 0:
                                nc.vector.tensor_mul(e_t[:], e_t[:], mask_sb[:, r, :])
                            # denominator partials accumulate on DVE (bf16)
                            if j == 0:
                                nc.vector.tensor_copy(dacc[:], e_t[:])
                            else:
                                nc.vector.tensor_add(dacc[:], dacc[:], e_t[:])
                            nc.tensor.matmul(outT_ps[:], v_sb[:, j, h * HD:(h + 1) * HD],
                                             e_t[:], start=(j == 0), stop=(j == n_k - 1))
                        # per-head epilogue, fully independent of the other
                        # head and free of any DMA hop (a tiny SBUF->SBUF dma
                        # here would queue behind megabytes of bulk x/wo
                        # descriptors): fold dacc -> [1,NS] den psum row,
                        # approx-reciprocal, broadcast, normalize, ship out.
                        den_ps = ps_den.tile([1, NS], F32, tag="den")
                        nc.tensor.matmul(den_ps[:], ones_sb[:], dacc[:],
                                         start=True, stop=True)
                        aof = smallp.tile([P, NS], BF16, tag="aof", bufs=2,
                                          name="aof")
                        nc.vector.tensor_copy(aof[:], outT_ps[:])
                        den_rec = smallp.tile([1, NS], F32, tag="den_rec", bufs=1)
                        nc.vector.reciprocal_approx_fast(den_rec[:], den_ps[:])
                        den_bc = smallp.tile([P, NS], F32, tag="den_bc")
                        nc.gpsimd.partition_broadcast(den_bc[:], den_rec[:])
                        ao = smallp.tile([P, NS], BF16, tag="ao")
                        nc.vector.tensor_mul(ao[:], aof[:], den_bc[:])
                        dst = a2a_in[b][2 * i:2 * i + 2, :, h, :]
                        nc.sync.dma_start(dst.transpose([1, 0, 2]),
                                          ao[:].rearrange("p (g q) -> p g q", g=2))
                    # out-projection of the PREVIOUS batch, one feature span
                    # per attention span: ready PE filler for the exp-paced
                    # attention window, at lower scheduler priority. For the
                    # last batch, outproj(B-2) is deferred to the tail so its
                    # matmuls hide the a2a(B-1) peer-skew wait (the scheduler
                    # still backfills attention(B-1) slack with them).
                    if 0 < b < B - 1:
                        outproj_nf(b - 1, i)

                # ---------------- head -> sequence redistribution ---------
                nc.gpsimd.collective_compute(
                    "AllToAll", mybir.AluOpType.bypass,
                    replica_groups=[list(range(N_CORES))],
                    ins=[a2a_in[b][:].opt()], outs=[a2a_out[b][:].opt()])

            for nf in range(NF):
                outproj_nf(B - 2, nf)
            for nf in range(NF):
                outproj_nf(B - 1, nf)
            flush_evicts()

    nc.compile()
    return nc


def _get_nc():
    global _cached_nc
    if _cached_nc is None:
        _cached_nc = _build()
    return _cached_nc


def prep_in_maps(query, key, value, Wq, Wk, Wv, Wo, bo):
    """Host-side layout prep matching the dram_tensor declarations: every
    tensor is pre-arranged so each DMA reads contiguous per-partition runs."""
    bf = ml_dtypes.bfloat16

    def act_prep(x):
        # [B, S, E] -> [B, NBLK, P, EC, NS]; (b, blk, p, ec, ns) =
        # x.T[b, ec*P + p, blk*NS + ns]
        xt = np.asarray(x, dtype=np.float32).transpose(0, 2, 1)
        xt = xt.reshape(B, EC, P, NBLK, NS).transpose(0, 3, 2, 1, 4)
        return np.ascontiguousarray(xt).astype(bf)

    qt = act_prep(query)
    kt = act_prep(key)
    vt = act_prep(value)
    wot = np.asarray(Wo, dtype=np.float32).T.reshape(EC, P, NF, NS)
    wot = np.ascontiguousarray(wot.transpose(2, 1, 0, 3)).astype(bf)
    bias_bc = np.broadcast_to(np.asarray(bo, np.float32), (P, E)).astype(bf)

    # causal masks for the 4 diagonal shifts: mask_r[kk, qq] = kk <= qq - 128 r
    kk = np.arange(P)[:, None]
    qq = np.arange(NS)[None, :]
    masks = np.stack([(kk <= qq - P * r) for r in range(4)], axis=1).astype(bf)

    def w_prep(W, sl):
        wt = np.asarray(W, np.float32)[sl].T.reshape(EC, P, F_LOC)
        return np.ascontiguousarray(wt.transpose(1, 0, 2)).astype(bf)

    in_maps = []
    for c in range(N_CORES):
        sl = slice(c * F_LOC, (c + 1) * F_LOC)
        in_maps.append(dict(
            qt=qt, kt=kt, vt=vt,
            wqt=w_prep(Wq, sl), wkt=w_prep(Wk, sl), wvt=w_prep(Wv, sl),
            wot=wot, bias_bc=bias_bc, masks=masks,
        ))
    return in_maps


def kernel(query, key, value, key_padding_mask, Wq, Wk, Wv, Wo, bo):
    in_maps = prep_in_maps(query, key, value, Wq, Wk, Wv, Wo, bo)
    nc = _get_nc()
    res = bass_utils.run_bass_kernel_spmd(
        nc, in_maps, core_ids=list(range(N_CORES)), trace=False)

    out = np.empty((B, S, E), dtype=np.float32)
    for c in range(N_CORES):
        out[:, c * S_LOC:(c + 1) * S_LOC, :] = res.results[c]["out"]
    return out


# revision 20
# speedup vs baseline: 1.4611x; 1.0293x over previous
"""Causal multi-head attention on 8 Trainium2 NeuronCores.

Problem: B=4, S=2048, E=2048, H=16 heads (HD=128), fp32 I/O.

Sharding (tensor-parallel on heads + sequence-parallel out-proj):
  - Every core holds the full (host-transposed, bf16-cast) activations and
    projects Q/K/V only for its 2 heads (per-core slices of Wq/Wk/Wv rows).
  - Attention (scores -> exp -> normalize -> @V) runs fully local per
    (batch, head), producing attn_outT [d_local=256, s=2048] per batch.
  - An AllToAll redistributes attn_outT from head-sharded to
    sequence-sharded: core c ends with attn_outT [e=2048, s_c=256] per batch.
  - Out-projection is computed for the core's 256 sequence rows per batch;
    the host concatenates row-slices - no further reduction needed.

v2 design notes (from perfetto analysis of v1 @ 1011us):
  - The PE only reaches its full 2.4GHz clock after ~3us of continuous
    execution; every idle gap drops it to 1.2GHz. So the whole kernel is
    organized to keep the PE stream dense:
      * proj uses one mega-DMA [128,16,512] per 512-col block (sync queue,
        ~12 issues/batch instead of ~96) and 2 serially-accumulated psum
        banks, leaving banks for overlap.
      * psum pools: scores 2 + attn-acc 2 + den 1 + proj/outproj 3 = 8.
      * outproj(b-1) is emitted interleaved after each attention span of
        batch b, so the out-of-order Tile scheduler uses its matmuls to
        fill PE slack in the EXP-paced attention phase. proj(b+1) fills
        whatever is left (it is emitted later = lower priority).
  - The v1 softmax epilogue (DVE reciprocal 3.3us + den DMA + ao-mul all
    serialized on the scalar queue) stalled the exp stream ~5us/span-pair.
    Now: reciprocal_approx_fast (DVE, ~0.7us, 18-bit - plenty for bf16
    outputs), den row-1 hop + broadcasts + acc evictions on gpsimd, ao
    muls + output DMAs on the vector queue. The scalar queue runs exps
    back-to-back only.
  - Wo is no longer SBUF-resident (64KB/partition): it streams per 512-col
    slice during the attention window (DMA is idle there). Frees SBUF for
    x-block triple buffering.
"""

import numpy as np
import ml_dtypes

import concourse.bacc as bacc
import concourse.mybir as mybir
import concourse.tile as tile
import concourse.bass_utils as bass_utils
from concourse.masks import make_identity

B, S, E, H = 4, 2048, 2048, 16
HD = E // H            # 128
N_CORES = 8
H_LOC = H // N_CORES   # 2 heads per core
F_LOC = H_LOC * HD     # 256 features per core (head slice)
S_LOC = S // N_CORES   # 256 sequence rows per core (out-proj slice)
P = 128
NS = 512               # matmul free-dim span
EC = E // P            # 16 contraction chunks
QSP = S // NS          # 4 q-spans per (b, h)
KCH = S // P           # 16 k-chunks
NBLK = S // NS         # 4 proj blocks per tensor
NF = E // NS           # 4 out-proj feature spans
INV_SQRT_HD = float(1.0 / np.sqrt(HD))

BF16 = mybir.dt.bfloat16
F32 = mybir.dt.float32

_cached_nc = None


def _build():
    nc = bacc.Bacc("TRN2", target_bir_lowering=False, debug=False,
                   num_devices=N_CORES)

    # ---------------- I/O ----------------
    # all inputs are host-pre-arranged so every DMA reads contiguous
    # per-partition runs (x: 16KB, weights: 8KB, wo: 8KB) - 2KB+ descriptors
    # keep the 16 DMA engines at full rate.
    qt_d = nc.dram_tensor("qt", [B, NBLK, P, EC, NS], BF16, kind="ExternalInput")
    kt_d = nc.dram_tensor("kt", [B, NBLK, P, EC, NS], BF16, kind="ExternalInput")
    vt_d = nc.dram_tensor("vt", [B, NBLK, P, EC, NS], BF16, kind="ExternalInput")
    wqt_d = nc.dram_tensor("wqt", [P, EC, F_LOC], BF16, kind="ExternalInput")
    wkt_d = nc.dram_tensor("wkt", [P, EC, F_LOC], BF16, kind="ExternalInput")
    wvt_d = nc.dram_tensor("wvt", [P, EC, F_LOC], BF16, kind="ExternalInput")
    wot_d = nc.dram_tensor("wot", [NF, P, EC, NS], BF16, kind="ExternalInput")
    bias_d = nc.dram_tensor("bias_bc", [P, E], BF16, kind="ExternalInput")
    masks_d = nc.dram_tensor("masks", [P, 4, NS], BF16, kind="ExternalInput")
    out_d = nc.dram_tensor("out", [B, S_LOC, E], F32, kind="ExternalOutput")

    with tile.TileContext(nc) as tc:
        with (
            tc.tile_pool(name="wconst", bufs=1) as wconst,
            tc.tile_pool(name="proj", bufs=2) as proj,
            tc.tile_pool(name="xs", bufs=3) as xs,
            tc.tile_pool(name="wop", bufs=3) as wop,
            tc.tile_pool(name="lhsp", bufs=2) as lhsp,
            tc.tile_pool(name="expp", bufs=8) as expp,
            tc.tile_pool(name="smallp", bufs=2) as smallp,
            tc.tile_pool(name="outp", bufs=2) as outp,
            tc.tile_pool(name="ps_s", bufs=2, space="PSUM") as ps_s,
            tc.tile_pool(name="ps_acc", bufs=2, space="PSUM") as ps_acc,
            tc.tile_pool(name="ps_den", bufs=1, space="PSUM") as ps_den,
            tc.tile_pool(name="ps_po", bufs=3, space="PSUM") as ps_po,
            tc.tile_pool(name="dram", bufs=1, space="DRAM") as dram,
        ):
            # ------------ constants / weights resident in SBUF ------------
            # wq + the first x block land first (the startup critical path);
            # wk/wv dmas are emitted just before their first use inside the
            # b==0 projection loop.
            wq_sb = wconst.tile([P, EC, F_LOC], BF16, tag="wq")
            wk_sb = wconst.tile([P, EC, F_LOC], BF16, tag="wk")
            wv_sb = wconst.tile([P, EC, F_LOC], BF16, tag="wv")
            nc.sync.dma_start(wq_sb[:], wqt_d.ap())
            bias_sb = wconst.tile([P, E], BF16, tag="bias")
            nc.scalar.dma_start(bias_sb[:], bias_d.ap())
            mask_sb = wconst.tile([P, 4, NS], BF16, tag="mask")
            nc.scalar.dma_start(mask_sb[:], masks_d.ap())
            # all-ones [P, 1] lhsT: folds the 128 k-partitions of a dacc tile
            # into a single psum row (the per-head softmax denominator).
            ones_sb = wconst.tile([P, 1], BF16, tag="ones")
            nc.vector.memset(ones_sb[:], 1.0)
            ident_sb = wconst.tile([P, P], BF16, tag="ident")
            make_identity(nc, ident_sb[:])

            # per-(batch, head) a2a tiles: two half-size collectives per
            # batch. h0's collective triggers before the last span's h1
            # compute, and outproj starts its h0 half of the e-contraction
            # while h1 is still in flight - hides peer skew + transfer.
            a2a_in = [[dram.tile([N_CORES, P, S_LOC], BF16,
                                 tag=f"a2a_in{b}_{h}", name=f"a2a_in{b}_{h}")
                       for h in range(H_LOC)] for b in range(B)]
            a2a_out = [[dram.tile([N_CORES, P, S_LOC], BF16,
                                  tag=f"a2a_out{b}_{h}", name=f"a2a_out{b}_{h}")
                        for h in range(H_LOC)] for b in range(B)]

            # out-proj lhs tiles, keyed by batch (loaded lazily at nf==0);
            # pending (psum, b, nf, sc) evictions, flushed one span later so
            # the DVE never queue-blocks on in-flight filler matmuls.
            lhs_tiles = {}
            pending_evict = []

            def flush_evicts():
                while pending_evict:
                    ps, eb, enf, esc = pending_evict.pop(0)
                    o_t = outp.tile([P, NS], F32, tag="o", name="o_t")
                    nc.vector.tensor_add(o_t[:], ps[:],
                                         bias_sb[:, enf * NS:(enf + 1) * NS])
                    nc.gpsimd.dma_start(
                        out_d.ap()[eb, esc * P:(esc + 1) * P,
                                   enf * NS:(enf + 1) * NS],
                        o_t[:])

            def outproj_nf(b, nf):
                """One 512-wide feature span of batch b's out-projection.

                Emitted interleaved between attention spans of batch b+1 so
                its matmuls fill PE slack there. Evictions are deferred to
                the next call (the matmuls have completed by then).
                """
                flush_evicts()
                if nf == 0:
                    lts = []
                    for dc in range(H_LOC):
                        lt = lhsp.tile([P, N_CORES, S_LOC], BF16,
                                       tag=f"lo{dc}", bufs=1, name=f"lo{dc}")
                        nc.gpsimd.dma_start(
                            lt[:], a2a_out[b][dc][:].rearrange("r p s -> p r s"))
                        lts.append(lt)
                    lhs_tiles[b] = lts
                wo_t = wop.tile([P, EC, NS], BF16, tag="wo")
                nc.sync.dma_start(wo_t[:], wot_d.ap()[nf])
                for sc in range(S_LOC // P):
                    ps = ps_po.tile([P, NS], F32, tag="po", name="ops")
                    for dc in range(H_LOC):
                        for r in range(N_CORES):
                            nc.tensor.matmul(
                                ps[:],
                                lhs_tiles[b][dc][:, r, sc * P:(sc + 1) * P],
                                wo_t[:, H_LOC * r + dc, :],
                                start=(dc == 0 and r == 0),
                                stop=(dc == H_LOC - 1 and r == N_CORES - 1))
                    pending_evict.append((ps, b, nf, sc))

            for b in range(B):
                # -------- Q/K/V projections, all in T-layout [d, s] -------
                # One mega-DMA [128, EC, 512] per 512-col block; per head a
                # single psum bank accumulates serially over all 16 e-chunks.
                qT_sb = proj.tile([P, H_LOC, S], BF16, tag="qT")
                kT_sb = proj.tile([P, H_LOC, S], BF16, tag="kT")
                vT_sb = proj.tile([P, H_LOC, S], BF16, tag="vT", bufs=1)
                v_sb = proj.tile([P, KCH, F_LOC], BF16, tag="v", bufs=1)

                for src_d, w_sb, dst, wsrc_d in (
                        (qt_d, wq_sb, qT_sb, None),
                        (kt_d, wk_sb, kT_sb, wkt_d),
                        (vt_d, wv_sb, vT_sb, wvt_d)):
                    if b == 0 and wsrc_d is not None:
                        nc.sync.dma_start(w_sb[:], wsrc_d.ap())
                    for blk in range(NBLK):
                        x_t = xs.tile([P, EC, NS], BF16, tag="x")
                        nc.sync.dma_start(x_t[:], src_d.ap()[b, blk])
                        for h in range(H_LOC):
                            ps = ps_po.tile([P, NS], F32, tag="po", name="pps")
                            for ec in range(EC):
                                nc.tensor.matmul(
                                    ps[:], w_sb[:, ec, h * HD:(h + 1) * HD],
                                    x_t[:, ec, :],
                                    start=(ec == 0), stop=(ec == EC - 1))
                            nc.vector.tensor_copy(
                                dst[:, h, blk * NS:(blk + 1) * NS], ps[:])

                # v [s, d] from vT via PE transposes
                for sc in range(KCH):
                    for h in range(H_LOC):
                        tps = ps_po.tile([P, P], BF16, tag="po", name="tps")
                        nc.tensor.transpose(tps[:], vT_sb[:, h, sc * P:(sc + 1) * P],
                                            ident_sb[:])
                        nc.vector.tensor_copy(v_sb[:, sc, h * HD:(h + 1) * HD],
                                              tps[:])

                # ----- attention: q-span outer, head inner; the two heads'
                # denominators pack into one [2, NS] psum via one-hot lhsT ----
                for i in range(QSP):
                    n_k = 4 * i + 4
                    for h in range(H_LOC):
                        outT_ps = ps_acc.tile([P, NS], F32, tag="acc",
                                              name=f"acc{h}")
                        dacc = expp.tile([P, NS], BF16, tag="dacc", bufs=2)
                        for j in range(n_k):
                            s_ps = ps_s.tile([P, NS], F32, tag="s")
                            nc.tensor.matmul(
                                s_ps[:], kT_sb[:, h, j * P:(j + 1) * P],
                                qT_sb[:, h, i * NS:(i + 1) * NS],
                                start=True, stop=True)
                            e_t = expp.tile([P, NS], BF16, tag="e", bufs=7)
                            nc.scalar.activation(e_t[:], s_ps[:],
                                                 mybir.ActivationFunctionType.Exp,
                                                 scale=INV_SQRT_HD)
                            r = j - 4 * i
                            if r >= 0:
                                nc.vector.tensor_mul(e_t[:], e_t[:], mask_sb[:, r, :])
                            # denominator partials accumulate on DVE (bf16)
                            if j == 0:
                                nc.vector.tensor_copy(dacc[:], e_t[:])
                            else:
                                nc.vector.tensor_add(dacc[:], dacc[:], e_t[:])
                            nc.tensor.matmul(outT_ps[:], v_sb[:, j, h * HD:(h + 1) * HD],
                                             e_t[:], start=(j == 0), stop=(j == n_k - 1))
                        # per-head epilogue, fully independent of the other
                        # head and free of any DMA hop (a tiny SBUF->SBUF dma
                        # here would queue behind megabytes of bulk x/wo
                        # descriptors): fold dacc -> [1,NS] den psum row,
                        # approx-reciprocal, broadcast, normalize, ship out.
                        den_ps = ps_den.tile([1, NS], F32, tag="den")
                        nc.tensor.matmul(den_ps[:], ones_sb[:], dacc[:],
                                         start=True, stop=True)
                        aof = smallp.tile([P, NS], BF16, tag="aof", bufs=2,
                                          name="aof")
                        nc.vector.tensor_copy(aof[:], outT_ps[:])
                        den_rec = smallp.tile([1, NS], F32, tag="den_rec", bufs=1)
                        nc.vector.reciprocal_approx_fast(den_rec[:], den_ps[:])
                        den_bc = smallp.tile([P, NS], F32, tag="den_bc")
                        nc.gpsimd.partition_broadcast(den_bc[:], den_rec[:])
                        ao = smallp.tile([P, NS], BF16, tag="ao")
                        nc.vector.tensor_mul(ao[:], aof[:], den_bc[:])
                        dst = a2a_in[b][h][2 * i:2 * i + 2, :, :]
                        nc.sync.dma_start(dst.transpose([1, 0, 2]),
                                          ao[:].rearrange("p (g q) -> p g q", g=2))
                        # trigger this head's AllToAll as soon as its last
                        # span is shipped (h0's overlaps h1's compute)
                        if i == QSP - 1:
                            nc.gpsimd.collective_compute(
                                "AllToAll", mybir.AluOpType.bypass,
                                replica_groups=[list(range(N_CORES))],
                                ins=[a2a_in[b][h][:].opt()],
                                outs=[a2a_out[b][h][:].opt()])
                    # out-projection of the PREVIOUS batch, one feature span
                    # per attention span: ready PE filler for the exp-paced
                    # attention window, at lower scheduler priority. For the
                    # last batch, outproj(B-2) is deferred to the tail so its
                    # matmuls hide the a2a(B-1) peer-skew wait (the scheduler
                    # still backfills attention(B-1) slack with them).
                    if 0 < b < B - 1:
                        outproj_nf(b - 1, i)

            for nf in range(NF):
                outproj_nf(B - 2, nf)
            for nf in range(NF):
                outproj_nf(B - 1, nf)
            flush_evicts()

    nc.compile()
    return nc


def _get_nc():
    global _cached_nc
    if _cached_nc is None:
        _cached_nc = _build()
    return _cached_nc


def prep_in_maps(query, key, value, Wq, Wk, Wv, Wo, bo):
    """Host-side layout prep matching the dram_tensor declarations: every
    tensor is pre-arranged so each DMA reads contiguous per-partition runs."""
    bf = ml_dtypes.bfloat16

    def act_prep(x):
        # [B, S, E] -> [B, NBLK, P, EC, NS]; (b, blk, p, ec, ns) =
        # x.T[b, ec*P + p, blk*NS + ns]
        xt = np.asarray(x, dtype=np.float32).transpose(0, 2, 1)
        xt = xt.reshape(B, EC, P, NBLK, NS).transpose(0, 3, 2, 1, 4)
        return np.ascontiguousarray(xt).astype(bf)

    qt = act_prep(query)
    kt = act_prep(key)
    vt = act_prep(value)
    wot = np.asarray(Wo, dtype=np.float32).T.reshape(EC, P, NF, NS)
    wot = np.ascontiguousarray(wot.transpose(2, 1, 0, 3)).astype(bf)
    bias_bc = np.broadcast_to(np.asarray(bo, np.float32), (P, E)).astype(bf)

    # causal masks for the 4 diagonal shifts: mask_r[kk, qq] = kk <= qq - 128 r
    kk = np.arange(P)[:, None]
    qq = np.arange(NS)[None, :]
    masks = np.stack([(kk <= qq - P * r) for r in range(4)], axis=1).astype(bf)

    def w_prep(W, sl):
        wt = np.asarray(W, np.float32)[sl].T.reshape(EC, P, F_LOC)
        return np.ascontiguousarray(wt.transpose(1, 0, 2)).astype(bf)

    in_maps = []
    for c in range(N_CORES):
        sl = slice(c * F_LOC, (c + 1) * F_LOC)
        in_maps.append(dict(
            qt=qt, kt=kt, vt=vt,
            wqt=w_prep(Wq, sl), wkt=w_prep(Wk, sl), wvt=w_prep(Wv, sl),
            wot=wot, bias_bc=bias_bc, masks=masks,
        ))
    return in_maps


def kernel(query, key, value, key_padding_mask, Wq, Wk, Wv, Wo, bo):
    in_maps = prep_in_maps(query, key, value, Wq, Wk, Wv, Wo, bo)
    nc = _get_nc()
    res = bass_utils.run_bass_kernel_spmd(
        nc, in_maps, core_ids=list(range(N_CORES)), trace=False)

    out = np.empty((B, S, E), dtype=np.float32)
    for c in range(N_CORES):
        out[:, c * S_LOC:(c + 1) * S_LOC, :] = res.results[c]["out"]
    return out
